# revision 25
# baseline (speedup 1.0000x reference)
"""DeepSeek-MLA forward kernel for 8 Trainium2 NeuronCores (Bass/Tile).

Sharding: core c -> batch b = c // 4, head-group g = c % 4 (4 of 16 heads).
Each core computes its batch's down-projections (replicated x4 within the
batch group), its 4 heads' attention, and a partial output projection
y_part = out_heads_local @ w_o_local (stored bf16).  The host sums the 4
partials per batch (fp32) and stacks the 2 batches.

v2 design notes (vs the 344us baseline):
- P1 is k-chunk-outer: xT is loaded once as 16 [128, S] chunks (4KB DMA
  rows), accumulating all 4 s-blocks x {kv,q} in 8 PSUM banks, so the PE
  streams at DMA arrival rate with no re-loads.
- rmsnorm sum-of-squares via a ones[128x128] matmul (output broadcast to
  all partitions), reciprocal on DVE (reciprocal_approx_fast), sqrt on ACT.
  Scalar engine table loads: Sqrt once, then Exp once - no thrashing.
- P3 scores are computed per 2-head pair into [128, 2*SB] PSUM tiles so
  exp runs as one ACT op per pair (amortizes the ~200-cycle ACT overhead).
  The attn@v matmuls for k-tile i are emitted after the scores of k-tile
  i+1 (software pipelining) so the PE never waits on exp.
- Softmax epilogue: DVE reciprocal + gpsimd partition_broadcast + DVE mul
  (no Ln/Exp activation-table swaps).
- P4 (output projection) is interleaved into the attention j-loop so its
  matmuls fill PE slack while ACT catches up; partials stored as bf16.
"""

import os
import sys

import numpy as np

for _p in ("/opt/trn_rl_repo", "/root/.axon_site/_ro/trn_rl_repo"):
    if os.path.isdir(_p) and _p not in sys.path:
        sys.path.insert(0, _p)

import concourse.bass as bass
import concourse.mybir as mybir
import concourse.tile as tile
from concourse import bacc

B, S, D, H, DN, DR, R = 2, 2048, 2048, 16, 32, 32, 128
HD = DN + DR  # 64
NCORES = 8
NH = 4          # heads per core
SB = 512        # s-block (psum bank width in f32)
NSB = S // SB   # 4
ST = 128        # s-tile
NST = S // ST   # 16
KC = 128        # contraction chunk
NKC = D // KC   # 16
VW = HD + 1     # v columns incl. ones column (65)
F32 = mybir.dt.float32
BF16 = mybir.dt.bfloat16


def _build_nc(causal: bool, use_mask: bool):
    nc = bacc.Bacc("TRN2", target_bir_lowering=False, debug=False,
                   num_devices=NCORES)

    xT = nc.dram_tensor("xT", [D, S], BF16, kind="ExternalInput").ap()
    wkv = nc.dram_tensor("wkv", [KC, D], BF16, kind="ExternalInput").ap()
    wq = nc.dram_tensor("wq", [KC, D], BF16, kind="ExternalInput").ap()
    kb = nc.dram_tensor("kb", [R, 2 * KC], BF16, kind="ExternalInput").ap()
    ksh = nc.dram_tensor("ksh", [R, 2 * KC], BF16, kind="ExternalInput").ap()
    qb = nc.dram_tensor("qb", [R, 2 * KC], BF16, kind="ExternalInput").ap()
    qsh = nc.dram_tensor("qsh", [R, 2 * KC], BF16, kind="ExternalInput").ap()
    uv = nc.dram_tensor("uv", [R, NH * HD], BF16, kind="ExternalInput").ap()
    wo = nc.dram_tensor("wo", [KC, 2 * D], BF16, kind="ExternalInput").ap()
    cosP = nc.dram_tensor("cosP", [128, S], BF16, kind="ExternalInput").ap()
    sinP = nc.dram_tensor("sinP", [128, S], BF16, kind="ExternalInput").ap()
    maskT = None
    if use_mask:
        maskT = nc.dram_tensor("maskT", [S, S], F32, kind="ExternalInput").ap()
    y = nc.dram_tensor("y", [S, D], BF16, kind="ExternalOutput").ap()

    AF = mybir.ActivationFunctionType
    ALU = mybir.AluOpType

    with tile.TileContext(nc) as tc:
        from contextlib import ExitStack
        with ExitStack() as ctx:
            stat = ctx.enter_context(tc.tile_pool(name="static", bufs=1))
            # persistent SBUF tensors
            ckvT = stat.tile([R, S], BF16, name="ckvT")
            cqT = stat.tile([R, S], BF16, name="cqT")
            kT01 = stat.tile([128, S], BF16, name="kT01")
            kT23 = stat.tile([128, S], BF16, name="kT23")
            qT01 = stat.tile([128, S], BF16, name="qT01")
            qT23 = stat.tile([128, S], BF16, name="qT23")
            v_sb = stat.tile([128, NST * NH * VW], BF16, name="v_sb")
            outT01 = stat.tile([128, S], BF16, name="outT01")
            outT23 = stat.tile([128, S], BF16, name="outT23")
            wkv_sb = stat.tile([KC, D], BF16, name="wkv_sb")
            wq_sb = stat.tile([KC, D], BF16, name="wq_sb")
            kb_sb = stat.tile([R, 2 * KC], BF16, name="kb_sb")
            ksh_sb = stat.tile([R, 2 * KC], BF16, name="ksh_sb")
            qb_sb = stat.tile([R, 2 * KC], BF16, name="qb_sb")
            qsh_sb = stat.tile([R, 2 * KC], BF16, name="qsh_sb")
            uv_sb = stat.tile([R, NH * HD], BF16, name="uv_sb")
            wo_sb = stat.tile([KC, 2 * D], BF16, name="wo_sb")
            cosP_sb = stat.tile([128, S], BF16, name="cosP_sb")
            sinP_sb = stat.tile([128, S], BF16, name="sinP_sb")
            ones_bb = stat.tile([128, 128], BF16, name="ones_bb")
            tri2_sb = stat.tile([128, 256], BF16, name="tri2_sb")
            onesf_sb = stat.tile([128, 64], F32, name="onesf_sb")

            # P1-critical loads first so the PE can start ASAP (weight
            # pieces interleaved with the first x chunks).

            # ---------------- Phase 1: c_kv^T, c_q^T + RMS norm ----------
            with tc.tile_pool(name="p1x", bufs=1) as p1x, \
                 tc.tile_pool(name="p1ps", bufs=8, space="PSUM") as p1ps, \
                 tc.tile_pool(name="p1t", bufs=3) as p1t:
                xch = [p1x.tile([128, S], BF16, name=f"xch{k}", tag=f"x{k}")
                       for k in range(NKC)]
                for k in range(NKC):
                    if k < 4:
                        pc = slice(k * SB, (k + 1) * SB)
                        nc.sync.dma_start(wkv_sb[:, pc], wkv[:, pc])
                        nc.sync.dma_start(wq_sb[:, pc], wq[:, pc])
                    nc.sync.dma_start(xch[k][:], xT[k * KC:(k + 1) * KC, :])
                # remaining static loads (after the P1-critical stream)
                nc.sync.dma_start(kb_sb[:], kb)
                nc.sync.dma_start(ksh_sb[:], ksh)
                nc.sync.dma_start(qb_sb[:], qb)
                nc.sync.dma_start(qsh_sb[:], qsh)
                nc.sync.dma_start(uv_sb[:], uv)
                nc.sync.dma_start(cosP_sb[:], cosP)
                nc.sync.dma_start(sinP_sb[:], sinP)
                nc.sync.dma_start(wo_sb[:], wo)
                nc.gpsimd.memset(ones_bb[:], 1.0)
                nc.gpsimd.memset(onesf_sb[:], 1.0)
                # tri[p, f] = 1.0 if p <= f else 0.0 (keep-lower-triangle
                # gate for diagonal score strips in k-major layout),
                # stored twice side by side so one DVE op covers a pair
                nc.gpsimd.memset(tri2_sb[:], 1.0)
                for _h in range(2):
                    nc.gpsimd.affine_select(
                        out=tri2_sb[:, _h * 128:(_h + 1) * 128],
                        in_=tri2_sb[:, _h * 128:(_h + 1) * 128],
                        compare_op=ALU.is_ge, fill=0.0, base=0,
                        channel_multiplier=-1, pattern=[[1, 128]])
                # ones column of v (col 64 of each 65-wide block)
                v_blocks = v_sb.rearrange("p (t h w) -> p t h w", t=NST, h=NH)
                nc.vector.tensor_copy(
                    v_blocks[:, :, :, HD:VW],
                    onesf_sb.rearrange("p (t h w) -> p t h w", t=NST, h=NH))

                cps = {}
                for sb in range(NSB):
                    for t, _ in ((0, None), (1, None)):
                        cps[(sb, t)] = p1ps.tile(
                            [128, SB], F32, name=f"cps{sb}_{t}", tag="cps")
                for k in range(NKC - 1):
                    for t, wsb in ((0, wkv_sb), (1, wq_sb)):
                        for sb in range(NSB):
                            nc.tensor.matmul(
                                cps[(sb, t)][:],
                                wsb[:, k * KC:(k + 1) * KC],
                                xch[k][:, sb * SB:(sb + 1) * SB],
                                start=(k == 0), stop=False)
                # last k-chunk and the rmsnorm drain go per s-block so
                # block 0's chain overlaps the other blocks' matmuls
                k = NKC - 1
                for sb in range(NSB):
                    sl = slice(sb * SB, (sb + 1) * SB)
                    for t, wsb in ((0, wkv_sb), (1, wq_sb)):
                        nc.tensor.matmul(
                            cps[(sb, t)][:],
                            wsb[:, k * KC:(k + 1) * KC],
                            xch[k][:, sl],
                            start=False, stop=True)
                    for t, cT in ((0, ckvT), (1, cqT)):
                        nc.scalar.activation(cT[:, sl], cps[(sb, t)][:],
                                             AF.Copy)
                        sqt = p1t.tile([128, SB], BF16, name="sqt", tag="sqt")
                        nc.vector.tensor_mul(sqt[:], cT[:, sl], cT[:, sl])
                        ms = p1ps.tile([128, SB], F32, name="ms", tag="cps")
                        nc.tensor.matmul(ms[:], ones_bb[:], sqt[:])
                        u = p1t.tile([128, SB], F32, name="u", tag="u")
                        nc.vector.reciprocal_approx_fast(u[:], ms[:])
                        rstd = p1t.tile([128, SB], F32, name="rstd", tag="rstd")
                        nc.scalar.activation(rstd[:], u[:], AF.Sqrt,
                                             scale=float(R))
                        nc.vector.tensor_mul(cT[:, sl], cT[:, sl], rstd[:])

            # ---------------- Phases 2+3+4 fused over s-blocks -----------
            with tc.tile_pool(name="psA", bufs=2, space="PSUM") as psA, \
                 tc.tile_pool(name="oa", bufs=4, space="PSUM") as oa, \
                 tc.tile_pool(name="etp", bufs=6) as etp, \
                 tc.tile_pool(name="vt", bufs=6) as vt, \
                 tc.tile_pool(name="yb", bufs=4) as yb, \
                 tc.tile_pool(name="ep", bufs=4) as ep, \
                 tc.tile_pool(name="mp", bufs=3) as mp:

                F32R = mybir.dt.float32r

                def emit_p4_tile(t4, dh):
                    yp = psA.tile([128, 2 * SB], F32, name="yp", tag="big")
                    for c, oT in ((0, outT01), (1, outT23)):
                        for half in range(2):
                            dlo = dh * 1024 + half * SB
                            nc.tensor.matmul(
                                yp[:, half * SB:(half + 1) * SB],
                                oT[:, t4 * ST:(t4 + 1) * ST],
                                wo_sb[:, c * D + dlo:c * D + dlo + SB],
                                start=(c == 0), stop=(c == 1))
                    ysb = yb.tile([128, 2 * SB], BF16, name="ysb", tag="y")
                    # split the PSUM drain across DVE and ACT
                    nc.vector.tensor_copy(ysb[:, 0:SB], yp[:, 0:SB])
                    nc.scalar.activation(ysb[:, SB:2 * SB], yp[:, SB:2 * SB],
                                         AF.Copy)
                    nc.sync.dma_start(
                        y[t4 * ST:(t4 + 1) * ST,
                          dh * 1024:(dh + 1) * 1024], ysb[:])

                def emit_p2(sb):
                    # up-projections + rope for s-block sb.  Pair tensors:
                    # rows [he_nope|he_rope|ho_nope|ho_rope]; cosP rows are
                    # 1.0 (sinP rows 0.0) on nope rows so one fused 3-op
                    # rope pass covers nope+rope together.
                    sl = slice(sb * SB, (sb + 1) * SB)
                    for cT, wb, wsh, dsts in (
                            (cqT, qb_sb, qsh_sb, (qT01, qT23)),
                            (ckvT, kb_sb, ksh_sb, (kT01, kT23))):
                        for p in range(2):
                            quad = psA.tile([128, 2 * SB], F32, name="p2q",
                                            tag="big")
                            nc.tensor.matmul(quad[:, 0:SB],
                                             wb[:, p * KC:(p + 1) * KC],
                                             cT[:, sl])
                            nc.tensor.matmul(quad[:, SB:2 * SB],
                                             wsh[:, p * KC:(p + 1) * KC],
                                             cT[:, sl])
                            t1 = vt.tile([128, SB], BF16, name="t1", tag="t")
                            t2 = vt.tile([128, SB], BF16, name="t2", tag="t")
                            nc.vector.tensor_mul(t1[:], quad[:, 0:SB],
                                                 cosP_sb[:, sl])
                            nc.vector.tensor_mul(t2[:], quad[:, SB:2 * SB],
                                                 sinP_sb[:, sl])
                            nc.vector.tensor_add(dsts[p][:, sl], t1[:], t2[:])
                    for t4 in range(4 * sb, 4 * sb + 4):
                        vq = psA.tile([128, 2 * SB], F32, name="vq", tag="big")
                        nc.tensor.matmul(vq[:, 0:NH * HD],
                                         ckvT[:, t4 * ST:(t4 + 1) * ST],
                                         uv_sb[:])
                        nc.vector.tensor_copy(
                            v_blocks[:, t4, :, 0:HD],
                            vq[:, 0:NH * HD].rearrange("p (h d) -> p h d",
                                                       h=NH))

                def emit_normalize(j, drained):
                    # 1/rowsum on DVE, broadcast via a tiny f32r ones-
                    # matmul on the PE (GpSimd library-reload latency is
                    # ~8us, so it must stay off this path), then scale.
                    oaS_l, dn_l = drained
                    for pp in range(2):
                        rbp = psA.tile([128, 2 * SB], F32, name="rbp",
                                       tag="big")
                        for hl in range(2):
                            hg = 2 * pp + hl
                            rc = ep.tile([1, SB], F32, name="rc", tag="rc")
                            nc.vector.reciprocal_approx_fast(
                                rc[:], dn_l[hg][:])
                            rcb = ep.tile([1, SB], BF16, name="rcb", tag="rcb")
                            nc.vector.tensor_copy(rcb[:], rc[:])
                            nc.tensor.matmul(
                                rbp[0:HD, hl * SB:(hl + 1) * SB],
                                ones_bb[0:1, 0:HD],
                                rcb[:])
                        dstT = (outT01, outT23)[pp]
                        for hl in range(2):
                            hg = 2 * pp + hl
                            nc.vector.tensor_mul(
                                dstT[hl * HD:(hl + 1) * HD,
                                     j * SB:(j + 1) * SB],
                                oaS_l[hg][:],
                                rbp[0:HD, hl * SB:(hl + 1) * SB])

                emit_p2(0)
                drained = None
                p4q = []
                for j in range(NSB):
                    ktiles = list(range(4 * j + 4)) if causal else \
                        list(range(NST))
                    oacc = [oa.tile([VW, SB], F32, name=f"oa{j}_{h}",
                                    tag="oa") for h in range(NH)]
                    ets = {}
                    q0s = {}

                    def emit_av(i):
                        q0 = q0s[i]
                        et = ets.pop(i)
                        for hg in range(NH):
                            p, hl = hg // 2, hg % 2
                            nc.tensor.matmul(
                                oacc[hg][:, q0:SB],
                                v_sb[:, i * (NH * VW) + hg * VW:
                                     i * (NH * VW) + (hg + 1) * VW],
                                et[p][:, hl * SB + q0:hl * SB + SB],
                                start=(i == ktiles[0]), stop=(i == ktiles[-1]))

                    n_k = len(ktiles)
                    for idx, i in enumerate(ktiles):
                        q0 = 128 * (i - 4 * j) if (causal and i >= 4 * j) else 0
                        q0s[i] = q0
                        mt = None
                        if use_mask:
                            mt = mp.tile([128, SB], F32, name="mt", tag="mt")
                            nc.sync.dma_start(
                                mt[:], maskT[i * 128:(i + 1) * 128,
                                             j * SB:(j + 1) * SB])
                        pair_et = []
                        for p in range(2):
                            kTp = (kT01, kT23)[p]
                            qTp = (qT01, qT23)[p]
                            scp = psA.tile([128, 2 * SB], F32, name="scp",
                                           tag="big")
                            nc.tensor.matmul(
                                scp[:, q0:SB],
                                kTp[0:64, i * 128:(i + 1) * 128],
                                qTp[0:64, j * SB + q0:(j + 1) * SB])
                            nc.tensor.matmul(
                                scp[:, SB + q0:2 * SB],
                                kTp[64:128, i * 128:(i + 1) * 128],
                                qTp[64:128, j * SB + q0:(j + 1) * SB])
                            if use_mask:
                                nc.vector.tensor_add(scp[:, 0:SB],
                                                     scp[:, 0:SB], mt[:])
                                nc.vector.tensor_add(scp[:, SB:2 * SB],
                                                     scp[:, SB:2 * SB], mt[:])
                            et = etp.tile([128, 2 * SB], BF16, name="et",
                                          tag="et")
                            if q0:
                                src = scp.rearrange(
                                    "p (b c) -> p b c", b=2)[:, :, q0:]
                                dst = et.rearrange(
                                    "p (b c) -> p b c", b=2)[:, :, q0:]
                            else:
                                src, dst = scp[:], et[:]
                            nc.scalar.activation(dst, src, AF.Exp, scale=0.125)
                            if causal and i >= 4 * j:
                                # one DVE op gates both heads' diagonal
                                # strips (tri2 holds the gate twice)
                                et_r = et.rearrange(
                                    "p (b c) -> p b c", b=2)[:, :, q0:q0 + 128]
                                nc.vector.tensor_mul(
                                    et_r, et_r,
                                    tri2_sb.rearrange("p (b c) -> p b c", b=2))
                            pair_et.append(et)
                        ets[i] = pair_et
                        # staggered cross-phase injections: each lands well
                        # before its consumers so boundaries never stall
                        if idx == 1 and j > 0:
                            emit_normalize(j - 1, drained)
                            p4q = [(t4, dh) for t4 in range(4 * (j - 1),
                                                           4 * (j - 1) + 4)
                                   for dh in range(2)]
                        if idx >= 2 and p4q:
                            # one output-projection tile per k-tile keeps
                            # the PE continuously busy (HAM stays warm)
                            emit_p4_tile(*p4q.pop(0))
                        if idx == n_k - 3 and j < NSB - 1:
                            emit_p2(j + 1)
                        # software pipeline: attn@v for the previous k-tile
                        if idx > 0:
                            emit_av(ktiles[idx - 1])
                    while p4q:
                        emit_p4_tile(*p4q.pop(0))
                    emit_av(ktiles[-1])

                    # drain oacc to SBUF immediately (partition-aligned
                    # copies) so the next j's accumulators never wait
                    oaS_l, dn_l = [], []
                    for hg in range(NH):
                        oaS = ep.tile([HD, SB], F32, name="oaS", tag="oaS")
                        nc.vector.tensor_copy(oaS[:], oacc[hg][0:HD, :])
                        dn = ep.tile([1, SB], F32, name="dn", tag="dn")
                        nc.scalar.activation(dn[:], oacc[hg][HD:VW, :],
                                             AF.Copy)
                        oaS_l.append(oaS)
                        dn_l.append(dn)
                    drained = (oaS_l, dn_l)

                emit_normalize(NSB - 1, drained)
                for t4 in range(4 * (NSB - 1), 4 * NSB):
                    for dh in range(2):
                        emit_p4_tile(t4, dh)

    nc.finalize()
    return nc


_NC_CACHE = {}


def _get_nc(causal, use_mask):
    key = (causal, use_mask)
    if key not in _NC_CACHE:
        _NC_CACHE[key] = _build_nc(causal, use_mask)
    return _NC_CACHE[key]


def _prep_inputs(x, cos, sin, mask, w_kv_down, kv_norm_w, w_uk, w_ur, w_uv,
                 w_q_down, q_norm_w, w_uq, w_qr, w_o, use_mask):
    """Build the 8 per-core input maps (host-side shard + fold)."""
    import ml_dtypes as md
    f = np.float32
    x = np.asarray(x, f)
    cos = np.asarray(cos, f)
    sin = np.asarray(sin, f)
    w_kv_down = np.asarray(w_kv_down, f)
    w_q_down = np.asarray(w_q_down, f)
    kv_norm_w = np.asarray(kv_norm_w, f)
    q_norm_w = np.asarray(q_norm_w, f)
    w_uk_e = np.asarray(w_uk, f) * kv_norm_w[:, None]
    w_ur_e = np.asarray(w_ur, f) * kv_norm_w[:, None]
    w_uv_e = np.asarray(w_uv, f) * kv_norm_w[:, None]
    w_uq_e = np.asarray(w_uq, f) * q_norm_w[:, None]
    w_qr_e = np.asarray(w_qr, f) * q_norm_w[:, None]
    w_o = np.asarray(w_o, f)

    # shared rearrangements
    wkv = np.ascontiguousarray(
        w_kv_down.reshape(NKC, KC, R).transpose(1, 0, 2).reshape(KC, D))
    wq = np.ascontiguousarray(
        w_q_down.reshape(NKC, KC, R).transpose(1, 0, 2).reshape(KC, D))
    cosT = np.ascontiguousarray(cos.T)                 # [32, S]
    sinT = np.ascontiguousarray(sin.T)
    sinSg = np.concatenate([-sinT[:DR // 2], sinT[DR // 2:]], axis=0)
    one32 = np.ones((DR, S), np.float32)
    zero32 = np.zeros((DR, S), np.float32)
    # pair-tensor rope tables: nope rows pass through (cos=1, sin=0)
    cosPt = np.ascontiguousarray(
        np.concatenate([one32, cosT, one32, cosT], axis=0)).astype(md.bfloat16)
    sinPt = np.ascontiguousarray(
        np.concatenate([zero32, sinSg, zero32, sinSg], axis=0)).astype(md.bfloat16)
    # rope shift permutation within each head's 32 cols
    perm = np.concatenate([np.arange(16, 32), np.arange(0, 16)])

    xTb = [np.ascontiguousarray(x[b].T).astype(md.bfloat16) for b in range(B)]
    maskT8 = None
    if use_mask:
        m = np.asarray(mask, f).reshape(S, S)
        maskT8 = np.ascontiguousarray(m.T) * 8.0

    in_maps = []
    z32 = np.zeros((R, DN), np.float32)
    for core in range(NCORES):
        b, g = core // 4, core % 4
        cs = slice(g * NH * DN, (g + 1) * NH * DN)      # 128-wide col slice
        vs = slice(g * NH * HD, (g + 1) * NH * HD)      # 256-wide
        uk_l = w_uk_e[:, cs].reshape(R, NH, DN)
        ur_l = w_ur_e[:, cs].reshape(R, NH, DR)
        urs_l = ur_l[:, :, perm]
        uq_l = w_uq_e[:, cs].reshape(R, NH, DN)
        qr_l = w_qr_e[:, cs].reshape(R, NH, DR)
        qrs_l = qr_l[:, :, perm]
        # pair layout: [he_nope | he_rope | ho_nope | ho_rope] per 128 cols
        def pair(nope, rope):
            cols = []
            for h in range(NH):
                cols += [nope[:, h], rope[:, h]]
            return np.ascontiguousarray(np.concatenate(cols, axis=1))
        def pair_sh(sh):
            cols = []
            for h in range(NH):
                cols += [z32, sh[:, h]]
            return np.ascontiguousarray(np.concatenate(cols, axis=1))
        wo_loc = w_o[g * NH * HD:(g + 1) * NH * HD]     # [256, D]
        wo_r = np.ascontiguousarray(
            wo_loc.reshape(2, KC, D).transpose(1, 0, 2).reshape(KC, 2 * D)
        ).astype(md.bfloat16)
        m_ = {
            "xT": xTb[b],
            "wkv": wkv.astype(md.bfloat16), "wq": wq.astype(md.bfloat16),
            "kb": pair(uk_l, ur_l).astype(md.bfloat16),
            "ksh": pair_sh(urs_l).astype(md.bfloat16),
            "qb": pair(uq_l, qr_l).astype(md.bfloat16),
            "qsh": pair_sh(qrs_l).astype(md.bfloat16),
            "uv": np.ascontiguousarray(w_uv_e[:, vs]).astype(md.bfloat16),
            "wo": wo_r,
            "cosP": cosPt, "sinP": sinPt,
        }
        if use_mask:
            m_["maskT"] = maskT8
        in_maps.append(m_)
    return in_maps


def _classify_mask(mask):
    m = np.asarray(mask, np.float32).reshape(S, S)
    if not np.any(m):
        return False, False          # dense, no mask
    causal_ref = np.where(
        np.tril(np.ones((S, S), dtype=bool)), np.float32(0.0),
        np.float32(-1e9))
    if np.array_equal(m, causal_ref):
        return True, False           # structural causal
    return False, True               # generic additive mask


LAST_RESULTS = None


def kernel(**inputs):
    global LAST_RESULTS
    from concourse.bass_utils import run_bass_kernel_spmd
    causal, use_mask = _classify_mask(inputs["mask"])
    nc = _get_nc(causal, use_mask)
    in_maps = _prep_inputs(
        inputs["x"], inputs["cos"], inputs["sin"], inputs["mask"],
        inputs["w_kv_down"], inputs["kv_norm_w"], inputs["w_uk"],
        inputs["w_ur"], inputs["w_uv"], inputs["w_q_down"],
        inputs["q_norm_w"], inputs["w_uq"], inputs["w_qr"], inputs["w_o"],
        use_mask)
    res = run_bass_kernel_spmd(nc, in_maps, list(range(NCORES)))
    LAST_RESULTS = res
    out = np.empty((B, S, D), np.float32)
    for b in range(B):
        acc = np.zeros((S, D), np.float32)
        for g in range(4):
            acc += np.asarray(res.results[4 * b + g]["y"]).astype(np.float32)
        out[b] = acc
    return out


# revision 27
# speedup vs baseline: 1.0141x; 1.0141x over previous
"""DeepSeek-MLA forward kernel for 8 Trainium2 NeuronCores (Bass/Tile).

Sharding: core c -> batch b = c // 4, head-group g = c % 4 (4 of 16 heads).
Each core computes its batch's down-projections (replicated x4 within the
batch group), its 4 heads' attention, and a partial output projection
y_part = out_heads_local @ w_o_local (stored bf16).  The host sums the 4
partials per batch (fp32) and stacks the 2 batches.

v2 design notes (vs the 344us baseline):
- P1 is k-chunk-outer: xT is loaded once as 16 [128, S] chunks (4KB DMA
  rows), accumulating all 4 s-blocks x {kv,q} in 8 PSUM banks, so the PE
  streams at DMA arrival rate with no re-loads.
- rmsnorm sum-of-squares via a ones[128x128] matmul (output broadcast to
  all partitions), reciprocal on DVE (reciprocal_approx_fast), sqrt on ACT.
  Scalar engine table loads: Sqrt once, then Exp once - no thrashing.
- P3 scores are computed per 2-head pair into [128, 2*SB] PSUM tiles so
  exp runs as one ACT op per pair (amortizes the ~200-cycle ACT overhead).
  The attn@v matmuls for k-tile i are emitted after the scores of k-tile
  i+1 (software pipelining) so the PE never waits on exp.
- Softmax epilogue: DVE reciprocal + gpsimd partition_broadcast + DVE mul
  (no Ln/Exp activation-table swaps).
- P4 (output projection) is interleaved into the attention j-loop so its
  matmuls fill PE slack while ACT catches up; partials stored as bf16.
"""

import os
import sys

import numpy as np

for _p in ("/opt/trn_rl_repo", "/root/.axon_site/_ro/trn_rl_repo"):
    if os.path.isdir(_p) and _p not in sys.path:
        sys.path.insert(0, _p)

import concourse.bass as bass
import concourse.mybir as mybir
import concourse.tile as tile
from concourse import bacc

B, S, D, H, DN, DR, R = 2, 2048, 2048, 16, 32, 32, 128
HD = DN + DR  # 64
NCORES = 8
NH = 4          # heads per core
SB = 512        # s-block (psum bank width in f32)
NSB = S // SB   # 4
ST = 128        # s-tile
NST = S // ST   # 16
KC = 128        # contraction chunk
NKC = D // KC   # 16
VW = HD + 1     # v columns incl. ones column (65)
F32 = mybir.dt.float32
BF16 = mybir.dt.bfloat16


def _build_nc(causal: bool, use_mask: bool):
    nc = bacc.Bacc("TRN2", target_bir_lowering=False, debug=False,
                   num_devices=NCORES)

    xT = nc.dram_tensor("xT", [D, S], BF16, kind="ExternalInput").ap()
    wkv = nc.dram_tensor("wkv", [KC, D], BF16, kind="ExternalInput").ap()
    wq = nc.dram_tensor("wq", [KC, D], BF16, kind="ExternalInput").ap()
    kb = nc.dram_tensor("kb", [R, 2 * KC], BF16, kind="ExternalInput").ap()
    ksh = nc.dram_tensor("ksh", [R, 2 * KC], BF16, kind="ExternalInput").ap()
    qb = nc.dram_tensor("qb", [R, 2 * KC], BF16, kind="ExternalInput").ap()
    qsh = nc.dram_tensor("qsh", [R, 2 * KC], BF16, kind="ExternalInput").ap()
    uv = nc.dram_tensor("uv", [R, NH * HD], BF16, kind="ExternalInput").ap()
    wo = nc.dram_tensor("wo", [KC, 2 * D], BF16, kind="ExternalInput").ap()
    cosP = nc.dram_tensor("cosP", [128, S], BF16, kind="ExternalInput").ap()
    sinP = nc.dram_tensor("sinP", [128, S], BF16, kind="ExternalInput").ap()
    maskT = None
    if use_mask:
        maskT = nc.dram_tensor("maskT", [S, S], F32, kind="ExternalInput").ap()
    y = nc.dram_tensor("y", [S, D], BF16, kind="ExternalOutput").ap()

    AF = mybir.ActivationFunctionType
    ALU = mybir.AluOpType

    with tile.TileContext(nc) as tc:
        from contextlib import ExitStack
        with ExitStack() as ctx:
            stat = ctx.enter_context(tc.tile_pool(name="static", bufs=1))
            # persistent SBUF tensors
            ckvT = stat.tile([R, S], BF16, name="ckvT")
            cqT = stat.tile([R, S], BF16, name="cqT")
            kT01 = stat.tile([128, S], BF16, name="kT01")
            kT23 = stat.tile([128, S], BF16, name="kT23")
            qT01 = stat.tile([128, S], BF16, name="qT01")
            qT23 = stat.tile([128, S], BF16, name="qT23")
            v_sb = stat.tile([128, NST * NH * VW], BF16, name="v_sb")
            outT01 = stat.tile([128, S], BF16, name="outT01")
            outT23 = stat.tile([128, S], BF16, name="outT23")
            wkv_sb = stat.tile([KC, D], BF16, name="wkv_sb")
            wq_sb = stat.tile([KC, D], BF16, name="wq_sb")
            kb_sb = stat.tile([R, 2 * KC], BF16, name="kb_sb")
            ksh_sb = stat.tile([R, 2 * KC], BF16, name="ksh_sb")
            qb_sb = stat.tile([R, 2 * KC], BF16, name="qb_sb")
            qsh_sb = stat.tile([R, 2 * KC], BF16, name="qsh_sb")
            uv_sb = stat.tile([R, NH * HD], BF16, name="uv_sb")
            wo_sb = stat.tile([KC, 2 * D], BF16, name="wo_sb")
            cosP_sb = stat.tile([128, S], BF16, name="cosP_sb")
            sinP_sb = stat.tile([128, S], BF16, name="sinP_sb")
            ones_bb = stat.tile([128, 128], BF16, name="ones_bb")
            tri2_sb = stat.tile([128, 256], BF16, name="tri2_sb")
            onesf_sb = stat.tile([128, 64], F32, name="onesf_sb")

            # P1-critical loads first so the PE can start ASAP (weight
            # pieces interleaved with the first x chunks).

            # ---------------- Phase 1: c_kv^T, c_q^T + RMS norm ----------
            with tc.tile_pool(name="p1x", bufs=1) as p1x, \
                 tc.tile_pool(name="p1ps", bufs=8, space="PSUM") as p1ps, \
                 tc.tile_pool(name="p1t", bufs=3) as p1t:
                xch = [p1x.tile([128, S], BF16, name=f"xch{k}", tag=f"x{k}")
                       for k in range(NKC)]
                for k in range(NKC):
                    if k < 4:
                        pc = slice(k * SB, (k + 1) * SB)
                        nc.sync.dma_start(wkv_sb[:, pc], wkv[:, pc])
                        nc.sync.dma_start(wq_sb[:, pc], wq[:, pc])
                    nc.sync.dma_start(xch[k][:], xT[k * KC:(k + 1) * KC, :])
                # remaining static loads (after the P1-critical stream)
                nc.sync.dma_start(kb_sb[:], kb)
                nc.sync.dma_start(ksh_sb[:], ksh)
                nc.sync.dma_start(qb_sb[:], qb)
                nc.sync.dma_start(qsh_sb[:], qsh)
                nc.sync.dma_start(uv_sb[:], uv)
                nc.sync.dma_start(cosP_sb[:], cosP)
                nc.sync.dma_start(sinP_sb[:], sinP)
                nc.sync.dma_start(wo_sb[:], wo)
                nc.gpsimd.memset(ones_bb[:], 1.0)
                nc.gpsimd.memset(onesf_sb[:], 1.0)
                # tri[p, f] = 1.0 if p <= f else 0.0 (keep-lower-triangle
                # gate for diagonal score strips in k-major layout),
                # stored twice side by side so one DVE op covers a pair
                nc.gpsimd.memset(tri2_sb[:], 1.0)
                for _h in range(2):
                    nc.gpsimd.affine_select(
                        out=tri2_sb[:, _h * 128:(_h + 1) * 128],
                        in_=tri2_sb[:, _h * 128:(_h + 1) * 128],
                        compare_op=ALU.is_ge, fill=0.0, base=0,
                        channel_multiplier=-1, pattern=[[1, 128]])
                # ones column of v (col 64 of each 65-wide block)
                v_blocks = v_sb.rearrange("p (t h w) -> p t h w", t=NST, h=NH)
                nc.vector.tensor_copy(
                    v_blocks[:, :, :, HD:VW],
                    onesf_sb.rearrange("p (t h w) -> p t h w", t=NST, h=NH))

                cps = {}
                for sb in range(NSB):
                    for t, _ in ((0, None), (1, None)):
                        cps[(sb, t)] = p1ps.tile(
                            [128, SB], F32, name=f"cps{sb}_{t}", tag="cps")
                for k in range(NKC - 1):
                    for t, wsb in ((0, wkv_sb), (1, wq_sb)):
                        for sb in range(NSB):
                            nc.tensor.matmul(
                                cps[(sb, t)][:],
                                wsb[:, k * KC:(k + 1) * KC],
                                xch[k][:, sb * SB:(sb + 1) * SB],
                                start=(k == 0), stop=False)
                # last k-chunk and the rmsnorm drain go per s-block so
                # block 0's chain overlaps the other blocks' matmuls
                k = NKC - 1
                for sb in range(NSB):
                    sl = slice(sb * SB, (sb + 1) * SB)
                    for t, wsb in ((0, wkv_sb), (1, wq_sb)):
                        nc.tensor.matmul(
                            cps[(sb, t)][:],
                            wsb[:, k * KC:(k + 1) * KC],
                            xch[k][:, sl],
                            start=False, stop=True)
                    for t, cT in ((0, ckvT), (1, cqT)):
                        nc.scalar.activation(cT[:, sl], cps[(sb, t)][:],
                                             AF.Copy)
                        sqt = p1t.tile([128, SB], BF16, name="sqt", tag="sqt")
                        nc.vector.tensor_mul(sqt[:], cT[:, sl], cT[:, sl])
                        ms = p1ps.tile([128, SB], F32, name="ms", tag="cps")
                        nc.tensor.matmul(ms[:], ones_bb[:], sqt[:])
                        u = p1t.tile([128, SB], F32, name="u", tag="u")
                        nc.vector.reciprocal_approx_fast(u[:], ms[:])
                        rstd = p1t.tile([128, SB], F32, name="rstd", tag="rstd")
                        nc.scalar.activation(rstd[:], u[:], AF.Sqrt,
                                             scale=float(R))
                        nc.vector.tensor_mul(cT[:, sl], cT[:, sl], rstd[:])

            # ---------------- Phases 2+3+4 fused over s-blocks -----------
            with tc.tile_pool(name="psA", bufs=2, space="PSUM") as psA, \
                 tc.tile_pool(name="oa", bufs=4, space="PSUM") as oa, \
                 tc.tile_pool(name="etp", bufs=6) as etp, \
                 tc.tile_pool(name="vt", bufs=6) as vt, \
                 tc.tile_pool(name="yb", bufs=4) as yb, \
                 tc.tile_pool(name="ep", bufs=4) as ep, \
                 tc.tile_pool(name="mp", bufs=3) as mp:

                F32R = mybir.dt.float32r

                def emit_p4_tile(t4, dh):
                    yp = psA.tile([128, 2 * SB], F32, name="yp", tag="big")
                    for c, oT in ((0, outT01), (1, outT23)):
                        for half in range(2):
                            dlo = dh * 1024 + half * SB
                            nc.tensor.matmul(
                                yp[:, half * SB:(half + 1) * SB],
                                oT[:, t4 * ST:(t4 + 1) * ST],
                                wo_sb[:, c * D + dlo:c * D + dlo + SB],
                                start=(c == 0), stop=(c == 1))
                    ysb = yb.tile([128, 2 * SB], BF16, name="ysb", tag="y")
                    # split the PSUM drain across DVE and ACT
                    nc.vector.tensor_copy(ysb[:, 0:SB], yp[:, 0:SB])
                    nc.scalar.activation(ysb[:, SB:2 * SB], yp[:, SB:2 * SB],
                                         AF.Copy)
                    nc.sync.dma_start(
                        y[t4 * ST:(t4 + 1) * ST,
                          dh * 1024:(dh + 1) * 1024], ysb[:])

                def emit_p2_part(sb, part):
                    # one quarter of the up-projection + rope work for
                    # s-block sb (q pairs first, then kv), plus one v tile.
                    # Pair tensors: rows [he_nope|he_rope|ho_nope|ho_rope];
                    # cosP rows are 1.0 (sinP rows 0.0) on nope rows so one
                    # fused 3-op rope pass covers nope+rope together.
                    sl = slice(sb * SB, (sb + 1) * SB)
                    cT, wb, wsh, dsts = (
                        (cqT, qb_sb, qsh_sb, (qT01, qT23)),
                        (ckvT, kb_sb, ksh_sb, (kT01, kT23)))[part // 2]
                    p = part % 2
                    quad = psA.tile([128, 2 * SB], F32, name="p2q", tag="big")
                    nc.tensor.matmul(quad[:, 0:SB],
                                     wb[:, p * KC:(p + 1) * KC], cT[:, sl])
                    nc.tensor.matmul(quad[:, SB:2 * SB],
                                     wsh[:, p * KC:(p + 1) * KC], cT[:, sl])
                    t1 = vt.tile([128, SB], BF16, name="t1", tag="t")
                    t2 = vt.tile([128, SB], BF16, name="t2", tag="t")
                    nc.vector.tensor_mul(t1[:], quad[:, 0:SB], cosP_sb[:, sl])
                    nc.vector.tensor_mul(t2[:], quad[:, SB:2 * SB],
                                         sinP_sb[:, sl])
                    nc.vector.tensor_add(dsts[p][:, sl], t1[:], t2[:])
                    t4 = 4 * sb + part
                    vq = psA.tile([128, 2 * SB], F32, name="vq", tag="big")
                    nc.tensor.matmul(vq[:, 0:NH * HD],
                                     ckvT[:, t4 * ST:(t4 + 1) * ST], uv_sb[:])
                    nc.vector.tensor_copy(
                        v_blocks[:, t4, :, 0:HD],
                        vq[:, 0:NH * HD].rearrange("p (h d) -> p h d", h=NH))

                def emit_p2(sb):
                    for part in range(4):
                        emit_p2_part(sb, part)

                def emit_normalize(j, drained):
                    # 1/rowsum on DVE, broadcast via a tiny f32r ones-
                    # matmul on the PE (GpSimd library-reload latency is
                    # ~8us, so it must stay off this path), then scale.
                    oaS_l, dn_l = drained
                    for pp in range(2):
                        rbp = psA.tile([128, 2 * SB], F32, name="rbp",
                                       tag="big")
                        for hl in range(2):
                            hg = 2 * pp + hl
                            rc = ep.tile([1, SB], F32, name="rc", tag="rc")
                            nc.vector.reciprocal_approx_fast(
                                rc[:], dn_l[hg][:])
                            rcb = ep.tile([1, SB], BF16, name="rcb", tag="rcb")
                            nc.vector.tensor_copy(rcb[:], rc[:])
                            nc.tensor.matmul(
                                rbp[0:HD, hl * SB:(hl + 1) * SB],
                                ones_bb[0:1, 0:HD],
                                rcb[:])
                        dstT = (outT01, outT23)[pp]
                        for hl in range(2):
                            hg = 2 * pp + hl
                            nc.vector.tensor_mul(
                                dstT[hl * HD:(hl + 1) * HD,
                                     j * SB:(j + 1) * SB],
                                oaS_l[hg][:],
                                rbp[0:HD, hl * SB:(hl + 1) * SB])

                emit_p2(0)
                drained = None
                p4q = []
                for j in range(NSB):
                    ktiles = list(range(4 * j + 4)) if causal else \
                        list(range(NST))
                    oacc = [oa.tile([VW, SB], F32, name=f"oa{j}_{h}",
                                    tag="oa") for h in range(NH)]
                    ets = {}
                    q0s = {}

                    def emit_av(i):
                        q0 = q0s[i]
                        et = ets.pop(i)
                        for hg in range(NH):
                            p, hl = hg // 2, hg % 2
                            nc.tensor.matmul(
                                oacc[hg][:, q0:SB],
                                v_sb[:, i * (NH * VW) + hg * VW:
                                     i * (NH * VW) + (hg + 1) * VW],
                                et[p][:, hl * SB + q0:hl * SB + SB],
                                start=(i == ktiles[0]), stop=(i == ktiles[-1]))

                    n_k = len(ktiles)
                    for idx, i in enumerate(ktiles):
                        q0 = 128 * (i - 4 * j) if (causal and i >= 4 * j) else 0
                        q0s[i] = q0
                        mt = None
                        if use_mask:
                            mt = mp.tile([128, SB], F32, name="mt", tag="mt")
                            nc.sync.dma_start(
                                mt[:], maskT[i * 128:(i + 1) * 128,
                                             j * SB:(j + 1) * SB])
                        pair_et = []
                        for p in range(2):
                            kTp = (kT01, kT23)[p]
                            qTp = (qT01, qT23)[p]
                            scp = psA.tile([128, 2 * SB], F32, name="scp",
                                           tag="big")
                            nc.tensor.matmul(
                                scp[:, q0:SB],
                                kTp[0:64, i * 128:(i + 1) * 128],
                                qTp[0:64, j * SB + q0:(j + 1) * SB])
                            nc.tensor.matmul(
                                scp[:, SB + q0:2 * SB],
                                kTp[64:128, i * 128:(i + 1) * 128],
                                qTp[64:128, j * SB + q0:(j + 1) * SB])
                            if use_mask:
                                nc.vector.tensor_add(scp[:, 0:SB],
                                                     scp[:, 0:SB], mt[:])
                                nc.vector.tensor_add(scp[:, SB:2 * SB],
                                                     scp[:, SB:2 * SB], mt[:])
                            et = etp.tile([128, 2 * SB], BF16, name="et",
                                          tag="et")
                            if q0:
                                src = scp.rearrange(
                                    "p (b c) -> p b c", b=2)[:, :, q0:]
                                dst = et.rearrange(
                                    "p (b c) -> p b c", b=2)[:, :, q0:]
                            else:
                                src, dst = scp[:], et[:]
                            nc.scalar.activation(dst, src, AF.Exp, scale=0.125)
                            if causal and i >= 4 * j:
                                # one DVE op gates both heads' diagonal
                                # strips (tri2 holds the gate twice)
                                et_r = et.rearrange(
                                    "p (b c) -> p b c", b=2)[:, :, q0:q0 + 128]
                                nc.vector.tensor_mul(
                                    et_r, et_r,
                                    tri2_sb.rearrange("p (b c) -> p b c", b=2))
                            pair_et.append(et)
                        ets[i] = pair_et
                        # staggered cross-phase injections: each lands well
                        # before its consumers so boundaries never stall
                        if idx == 1 and j > 0:
                            emit_normalize(j - 1, drained)
                            p4q = [(t4, dh) for t4 in range(4 * (j - 1),
                                                           4 * (j - 1) + 4)
                                   for dh in range(2)]
                        if idx >= 2 and p4q:
                            # one output-projection tile per k-tile keeps
                            # the PE continuously busy (HAM stays warm)
                            emit_p4_tile(*p4q.pop(0))
                        if j < NSB - 1 and n_k - 6 <= idx <= n_k - 3:
                            if n_k >= 8:
                                emit_p2_part(j + 1, idx - (n_k - 6))
                            elif idx == n_k - 3:
                                emit_p2(j + 1)
                        # software pipeline: attn@v for the previous k-tile
                        if idx > 0:
                            emit_av(ktiles[idx - 1])
                    while p4q:
                        emit_p4_tile(*p4q.pop(0))
                    emit_av(ktiles[-1])

                    # drain oacc to SBUF immediately (partition-aligned
                    # copies) so the next j's accumulators never wait
                    oaS_l, dn_l = [], []
                    for hg in range(NH):
                        oaS = ep.tile([HD, SB], F32, name="oaS", tag="oaS")
                        nc.vector.tensor_copy(oaS[:], oacc[hg][0:HD, :])
                        dn = ep.tile([1, SB], F32, name="dn", tag="dn")
                        nc.scalar.activation(dn[:], oacc[hg][HD:VW, :],
                                             AF.Copy)
                        oaS_l.append(oaS)
                        dn_l.append(dn)
                    drained = (oaS_l, dn_l)

                emit_normalize(NSB - 1, drained)
                for t4 in range(4 * (NSB - 1), 4 * NSB):
                    for dh in range(2):
                        emit_p4_tile(t4, dh)

    nc.finalize()
    return nc


_NC_CACHE = {}


def _get_nc(causal, use_mask):
    key = (causal, use_mask)
    if key not in _NC_CACHE:
        _NC_CACHE[key] = _build_nc(causal, use_mask)
    return _NC_CACHE[key]


def _prep_inputs(x, cos, sin, mask, w_kv_down, kv_norm_w, w_uk, w_ur, w_uv,
                 w_q_down, q_norm_w, w_uq, w_qr, w_o, use_mask):
    """Build the 8 per-core input maps (host-side shard + fold)."""
    import ml_dtypes as md
    f = np.float32
    x = np.asarray(x, f)
    cos = np.asarray(cos, f)
    sin = np.asarray(sin, f)
    w_kv_down = np.asarray(w_kv_down, f)
    w_q_down = np.asarray(w_q_down, f)
    kv_norm_w = np.asarray(kv_norm_w, f)
    q_norm_w = np.asarray(q_norm_w, f)
    w_uk_e = np.asarray(w_uk, f) * kv_norm_w[:, None]
    w_ur_e = np.asarray(w_ur, f) * kv_norm_w[:, None]
    w_uv_e = np.asarray(w_uv, f) * kv_norm_w[:, None]
    w_uq_e = np.asarray(w_uq, f) * q_norm_w[:, None]
    w_qr_e = np.asarray(w_qr, f) * q_norm_w[:, None]
    w_o = np.asarray(w_o, f)

    # shared rearrangements
    wkv = np.ascontiguousarray(
        w_kv_down.reshape(NKC, KC, R).transpose(1, 0, 2).reshape(KC, D))
    wq = np.ascontiguousarray(
        w_q_down.reshape(NKC, KC, R).transpose(1, 0, 2).reshape(KC, D))
    cosT = np.ascontiguousarray(cos.T)                 # [32, S]
    sinT = np.ascontiguousarray(sin.T)
    sinSg = np.concatenate([-sinT[:DR // 2], sinT[DR // 2:]], axis=0)
    one32 = np.ones((DR, S), np.float32)
    zero32 = np.zeros((DR, S), np.float32)
    # pair-tensor rope tables: nope rows pass through (cos=1, sin=0)
    cosPt = np.ascontiguousarray(
        np.concatenate([one32, cosT, one32, cosT], axis=0)).astype(md.bfloat16)
    sinPt = np.ascontiguousarray(
        np.concatenate([zero32, sinSg, zero32, sinSg], axis=0)).astype(md.bfloat16)
    # rope shift permutation within each head's 32 cols
    perm = np.concatenate([np.arange(16, 32), np.arange(0, 16)])

    xTb = [np.ascontiguousarray(x[b].T).astype(md.bfloat16) for b in range(B)]
    maskT8 = None
    if use_mask:
        m = np.asarray(mask, f).reshape(S, S)
        maskT8 = np.ascontiguousarray(m.T) * 8.0

    in_maps = []
    z32 = np.zeros((R, DN), np.float32)
    for core in range(NCORES):
        b, g = core // 4, core % 4
        cs = slice(g * NH * DN, (g + 1) * NH * DN)      # 128-wide col slice
        vs = slice(g * NH * HD, (g + 1) * NH * HD)      # 256-wide
        uk_l = w_uk_e[:, cs].reshape(R, NH, DN)
        ur_l = w_ur_e[:, cs].reshape(R, NH, DR)
        urs_l = ur_l[:, :, perm]
        uq_l = w_uq_e[:, cs].reshape(R, NH, DN)
        qr_l = w_qr_e[:, cs].reshape(R, NH, DR)
        qrs_l = qr_l[:, :, perm]
        # pair layout: [he_nope | he_rope | ho_nope | ho_rope] per 128 cols
        def pair(nope, rope):
            cols = []
            for h in range(NH):
                cols += [nope[:, h], rope[:, h]]
            return np.ascontiguousarray(np.concatenate(cols, axis=1))
        def pair_sh(sh):
            cols = []
            for h in range(NH):
                cols += [z32, sh[:, h]]
            return np.ascontiguousarray(np.concatenate(cols, axis=1))
        wo_loc = w_o[g * NH * HD:(g + 1) * NH * HD]     # [256, D]
        wo_r = np.ascontiguousarray(
            wo_loc.reshape(2, KC, D).transpose(1, 0, 2).reshape(KC, 2 * D)
        ).astype(md.bfloat16)
        m_ = {
            "xT": xTb[b],
            "wkv": wkv.astype(md.bfloat16), "wq": wq.astype(md.bfloat16),
            "kb": pair(uk_l, ur_l).astype(md.bfloat16),
            "ksh": pair_sh(urs_l).astype(md.bfloat16),
            "qb": pair(uq_l, qr_l).astype(md.bfloat16),
            "qsh": pair_sh(qrs_l).astype(md.bfloat16),
            "uv": np.ascontiguousarray(w_uv_e[:, vs]).astype(md.bfloat16),
            "wo": wo_r,
            "cosP": cosPt, "sinP": sinPt,
        }
        if use_mask:
            m_["maskT"] = maskT8
        in_maps.append(m_)
    return in_maps


def _classify_mask(mask):
    m = np.asarray(mask, np.float32).reshape(S, S)
    if not np.any(m):
        return False, False          # dense, no mask
    causal_ref = np.where(
        np.tril(np.ones((S, S), dtype=bool)), np.float32(0.0),
        np.float32(-1e9))
    if np.array_equal(m, causal_ref):
        return True, False           # structural causal
    return False, True               # generic additive mask


LAST_RESULTS = None


def kernel(**inputs):
    global LAST_RESULTS
    from concourse.bass_utils import run_bass_kernel_spmd
    causal, use_mask = _classify_mask(inputs["mask"])
    nc = _get_nc(causal, use_mask)
    in_maps = _prep_inputs(
        inputs["x"], inputs["cos"], inputs["sin"], inputs["mask"],
        inputs["w_kv_down"], inputs["kv_norm_w"], inputs["w_uk"],
        inputs["w_ur"], inputs["w_uv"], inputs["w_q_down"],
        inputs["q_norm_w"], inputs["w_uq"], inputs["w_qr"], inputs["w_o"],
        use_mask)
    res = run_bass_kernel_spmd(nc, in_maps, list(range(NCORES)))
    LAST_RESULTS = res
    out = np.empty((B, S, D), np.float32)
    for b in range(B):
        acc = np.zeros((S, D), np.float32)
        for g in range(4):
            acc += np.asarray(res.results[4 * b + g]["y"]).astype(np.float32)
        out[b] = acc
    return out


# revision 28
# speedup vs baseline: 1.0399x; 1.0255x over previous
"""DeepSeek-MLA forward kernel for 8 Trainium2 NeuronCores (Bass/Tile).

Sharding: core c -> batch b = c // 4, head-group g = c % 4 (4 of 16 heads).
Each core computes its batch's down-projections (replicated x4 within the
batch group), its 4 heads' attention, and a partial output projection
y_part = out_heads_local @ w_o_local (stored bf16).  The host sums the 4
partials per batch (fp32) and stacks the 2 batches.

v2 design notes (vs the 344us baseline):
- P1 is k-chunk-outer: xT is loaded once as 16 [128, S] chunks (4KB DMA
  rows), accumulating all 4 s-blocks x {kv,q} in 8 PSUM banks, so the PE
  streams at DMA arrival rate with no re-loads.
- rmsnorm sum-of-squares via a ones[128x128] matmul (output broadcast to
  all partitions), reciprocal on DVE (reciprocal_approx_fast), sqrt on ACT.
  Scalar engine table loads: Sqrt once, then Exp once - no thrashing.
- P3 scores are computed per 2-head pair into [128, 2*SB] PSUM tiles so
  exp runs as one ACT op per pair (amortizes the ~200-cycle ACT overhead).
  The attn@v matmuls for k-tile i are emitted after the scores of k-tile
  i+1 (software pipelining) so the PE never waits on exp.
- Softmax epilogue: DVE reciprocal + gpsimd partition_broadcast + DVE mul
  (no Ln/Exp activation-table swaps).
- P4 (output projection) is interleaved into the attention j-loop so its
  matmuls fill PE slack while ACT catches up; partials stored as bf16.
"""

import os
import sys

import numpy as np

for _p in ("/opt/trn_rl_repo", "/root/.axon_site/_ro/trn_rl_repo"):
    if os.path.isdir(_p) and _p not in sys.path:
        sys.path.insert(0, _p)

import concourse.bass as bass
import concourse.mybir as mybir
import concourse.tile as tile
from concourse import bacc

B, S, D, H, DN, DR, R = 2, 2048, 2048, 16, 32, 32, 128
HD = DN + DR  # 64
NCORES = 8
NH = 4          # heads per core
SB = 512        # s-block (psum bank width in f32)
NSB = S // SB   # 4
ST = 128        # s-tile
NST = S // ST   # 16
KC = 128        # contraction chunk
NKC = D // KC   # 16
VW = HD + 1     # v columns incl. ones column (65)
F32 = mybir.dt.float32
BF16 = mybir.dt.bfloat16


def _build_nc(causal: bool, use_mask: bool):
    nc = bacc.Bacc("TRN2", target_bir_lowering=False, debug=False,
                   num_devices=NCORES)

    xT = nc.dram_tensor("xT", [D, S], BF16, kind="ExternalInput").ap()
    wkv = nc.dram_tensor("wkv", [KC, D], BF16, kind="ExternalInput").ap()
    wq = nc.dram_tensor("wq", [KC, D], BF16, kind="ExternalInput").ap()
    kb = nc.dram_tensor("kb", [R, 2 * KC], BF16, kind="ExternalInput").ap()
    ksh = nc.dram_tensor("ksh", [R, 2 * KC], BF16, kind="ExternalInput").ap()
    qb = nc.dram_tensor("qb", [R, 2 * KC], BF16, kind="ExternalInput").ap()
    qsh = nc.dram_tensor("qsh", [R, 2 * KC], BF16, kind="ExternalInput").ap()
    uv = nc.dram_tensor("uv", [R, NH * HD], BF16, kind="ExternalInput").ap()
    wo = nc.dram_tensor("wo", [KC, 2 * D], BF16, kind="ExternalInput").ap()
    cosP = nc.dram_tensor("cosP", [128, S], BF16, kind="ExternalInput").ap()
    sinP = nc.dram_tensor("sinP", [128, S], BF16, kind="ExternalInput").ap()
    maskT = None
    if use_mask:
        maskT = nc.dram_tensor("maskT", [S, S], F32, kind="ExternalInput").ap()
    y = nc.dram_tensor("y", [S, D], BF16, kind="ExternalOutput").ap()

    AF = mybir.ActivationFunctionType
    ALU = mybir.AluOpType

    with tile.TileContext(nc) as tc:
        from contextlib import ExitStack
        with ExitStack() as ctx:
            stat = ctx.enter_context(tc.tile_pool(name="static", bufs=1))
            # persistent SBUF tensors
            ckvT = stat.tile([R, S], BF16, name="ckvT")
            cqT = stat.tile([R, S], BF16, name="cqT")
            kT01 = stat.tile([128, S], BF16, name="kT01")
            kT23 = stat.tile([128, S], BF16, name="kT23")
            qT01 = stat.tile([128, S], BF16, name="qT01")
            qT23 = stat.tile([128, S], BF16, name="qT23")
            v_sb = stat.tile([128, NST * NH * VW], BF16, name="v_sb")
            outT01 = stat.tile([128, S], BF16, name="outT01")
            outT23 = stat.tile([128, S], BF16, name="outT23")
            wkv_sb = stat.tile([KC, D], BF16, name="wkv_sb")
            wq_sb = stat.tile([KC, D], BF16, name="wq_sb")
            kb_sb = stat.tile([R, 2 * KC], BF16, name="kb_sb")
            ksh_sb = stat.tile([R, 2 * KC], BF16, name="ksh_sb")
            qb_sb = stat.tile([R, 2 * KC], BF16, name="qb_sb")
            qsh_sb = stat.tile([R, 2 * KC], BF16, name="qsh_sb")
            uv_sb = stat.tile([R, NH * HD], BF16, name="uv_sb")
            wo_sb = stat.tile([KC, 2 * D], BF16, name="wo_sb")
            cosP_sb = stat.tile([128, S], BF16, name="cosP_sb")
            sinP_sb = stat.tile([128, S], BF16, name="sinP_sb")
            ones_bb = stat.tile([128, 128], BF16, name="ones_bb")
            tri2_sb = stat.tile([128, 256], BF16, name="tri2_sb")
            onesf_sb = stat.tile([128, 64], F32, name="onesf_sb")

            # P1-critical loads first so the PE can start ASAP (weight
            # pieces interleaved with the first x chunks).

            # ---------------- Phase 1: c_kv^T, c_q^T + RMS norm ----------
            with tc.tile_pool(name="p1x", bufs=1) as p1x, \
                 tc.tile_pool(name="p1ps", bufs=8, space="PSUM") as p1ps, \
                 tc.tile_pool(name="p1t", bufs=3) as p1t:
                xch = [p1x.tile([128, S], BF16, name=f"xch{k}", tag=f"x{k}")
                       for k in range(NKC)]
                for k in range(NKC):
                    if k < 4:
                        pc = slice(k * SB, (k + 1) * SB)
                        nc.sync.dma_start(wkv_sb[:, pc], wkv[:, pc])
                        nc.sync.dma_start(wq_sb[:, pc], wq[:, pc])
                    nc.sync.dma_start(xch[k][:], xT[k * KC:(k + 1) * KC, :])
                # remaining static loads (after the P1-critical stream)
                nc.sync.dma_start(kb_sb[:], kb)
                nc.sync.dma_start(ksh_sb[:], ksh)
                nc.sync.dma_start(qb_sb[:], qb)
                nc.sync.dma_start(qsh_sb[:], qsh)
                nc.sync.dma_start(uv_sb[:], uv)
                nc.sync.dma_start(cosP_sb[:], cosP)
                nc.sync.dma_start(sinP_sb[:], sinP)
                nc.sync.dma_start(wo_sb[:], wo)
                nc.gpsimd.memset(ones_bb[:], 1.0)
                nc.gpsimd.memset(onesf_sb[:], 1.0)
                # tri[p, f] = 1.0 if p <= f else 0.0 (keep-lower-triangle
                # gate for diagonal score strips in k-major layout),
                # stored twice side by side so one DVE op covers a pair
                nc.gpsimd.memset(tri2_sb[:], 1.0)
                for _h in range(2):
                    nc.gpsimd.affine_select(
                        out=tri2_sb[:, _h * 128:(_h + 1) * 128],
                        in_=tri2_sb[:, _h * 128:(_h + 1) * 128],
                        compare_op=ALU.is_ge, fill=0.0, base=0,
                        channel_multiplier=-1, pattern=[[1, 128]])
                # ones column of v (col 64 of each 65-wide block)
                v_blocks = v_sb.rearrange("p (t h w) -> p t h w", t=NST, h=NH)
                nc.vector.tensor_copy(
                    v_blocks[:, :, :, HD:VW],
                    onesf_sb.rearrange("p (t h w) -> p t h w", t=NST, h=NH))

                cps = {}
                for sb in range(NSB):
                    for t, _ in ((0, None), (1, None)):
                        cps[(sb, t)] = p1ps.tile(
                            [128, SB], F32, name=f"cps{sb}_{t}", tag="cps")
                for k in range(NKC - 1):
                    for t, wsb in ((0, wkv_sb), (1, wq_sb)):
                        for sb in range(NSB):
                            nc.tensor.matmul(
                                cps[(sb, t)][:],
                                wsb[:, k * KC:(k + 1) * KC],
                                xch[k][:, sb * SB:(sb + 1) * SB],
                                start=(k == 0), stop=False)
                # last k-chunk and the rmsnorm drain go per s-block so
                # block 0's chain overlaps the other blocks' matmuls
                k = NKC - 1
                for sb in range(NSB):
                    sl = slice(sb * SB, (sb + 1) * SB)
                    for t, wsb in ((0, wkv_sb), (1, wq_sb)):
                        nc.tensor.matmul(
                            cps[(sb, t)][:],
                            wsb[:, k * KC:(k + 1) * KC],
                            xch[k][:, sl],
                            start=False, stop=True)
                    for t, cT in ((0, ckvT), (1, cqT)):
                        nc.scalar.activation(cT[:, sl], cps[(sb, t)][:],
                                             AF.Copy)
                        sqt = p1t.tile([128, SB], BF16, name="sqt", tag="sqt")
                        nc.vector.tensor_mul(sqt[:], cT[:, sl], cT[:, sl])
                        ms = p1ps.tile([128, SB], F32, name="ms", tag="cps")
                        nc.tensor.matmul(ms[:], ones_bb[:], sqt[:])
                        u = p1t.tile([128, SB], F32, name="u", tag="u")
                        nc.vector.reciprocal_approx_fast(u[:], ms[:])
                        rstd = p1t.tile([128, SB], F32, name="rstd", tag="rstd")
                        nc.scalar.activation(rstd[:], u[:], AF.Sqrt,
                                             scale=float(R))
                        nc.vector.tensor_mul(cT[:, sl], cT[:, sl], rstd[:])

            # ---------------- Phases 2+3+4 fused over s-blocks -----------
            with tc.tile_pool(name="psA", bufs=2, space="PSUM") as psA, \
                 tc.tile_pool(name="oa", bufs=4, space="PSUM") as oa, \
                 tc.tile_pool(name="etp", bufs=6) as etp, \
                 tc.tile_pool(name="vt", bufs=6) as vt, \
                 tc.tile_pool(name="yb", bufs=4) as yb, \
                 tc.tile_pool(name="ep", bufs=4) as ep, \
                 tc.tile_pool(name="mp", bufs=3) as mp:

                F32R = mybir.dt.float32r

                def emit_p4_tile(t4, dh):
                    yp = psA.tile([128, 2 * SB], F32, name="yp", tag="big")
                    for c, oT in ((0, outT01), (1, outT23)):
                        for half in range(2):
                            dlo = dh * 1024 + half * SB
                            nc.tensor.matmul(
                                yp[:, half * SB:(half + 1) * SB],
                                oT[:, t4 * ST:(t4 + 1) * ST],
                                wo_sb[:, c * D + dlo:c * D + dlo + SB],
                                start=(c == 0), stop=(c == 1))
                    ysb = yb.tile([128, 2 * SB], BF16, name="ysb", tag="y")
                    # full drain on DVE: ACT must stay exp-only in steady
                    # state (a scalar-side copy here saturates ACT and
                    # stalls the score ring ~730ns every P4-carrying tile)
                    nc.vector.tensor_copy(ysb[:], yp[:])
                    nc.sync.dma_start(
                        y[t4 * ST:(t4 + 1) * ST,
                          dh * 1024:(dh + 1) * 1024], ysb[:])

                def emit_p2_part(sb, part):
                    # one quarter of the up-projection + rope work for
                    # s-block sb (q pairs first, then kv), plus one v tile.
                    # Pair tensors: rows [he_nope|he_rope|ho_nope|ho_rope];
                    # cosP rows are 1.0 (sinP rows 0.0) on nope rows so one
                    # fused 3-op rope pass covers nope+rope together.
                    sl = slice(sb * SB, (sb + 1) * SB)
                    cT, wb, wsh, dsts = (
                        (cqT, qb_sb, qsh_sb, (qT01, qT23)),
                        (ckvT, kb_sb, ksh_sb, (kT01, kT23)))[part // 2]
                    p = part % 2
                    quad = psA.tile([128, 2 * SB], F32, name="p2q", tag="big")
                    nc.tensor.matmul(quad[:, 0:SB],
                                     wb[:, p * KC:(p + 1) * KC], cT[:, sl])
                    nc.tensor.matmul(quad[:, SB:2 * SB],
                                     wsh[:, p * KC:(p + 1) * KC], cT[:, sl])
                    t1 = vt.tile([128, SB], BF16, name="t1", tag="t")
                    t2 = vt.tile([128, SB], BF16, name="t2", tag="t")
                    nc.vector.tensor_mul(t1[:], quad[:, 0:SB], cosP_sb[:, sl])
                    nc.vector.tensor_mul(t2[:], quad[:, SB:2 * SB],
                                         sinP_sb[:, sl])
                    nc.vector.tensor_add(dsts[p][:, sl], t1[:], t2[:])
                    t4 = 4 * sb + part
                    vq = psA.tile([128, 2 * SB], F32, name="vq", tag="big")
                    nc.tensor.matmul(vq[:, 0:NH * HD],
                                     ckvT[:, t4 * ST:(t4 + 1) * ST], uv_sb[:])
                    nc.vector.tensor_copy(
                        v_blocks[:, t4, :, 0:HD],
                        vq[:, 0:NH * HD].rearrange("p (h d) -> p h d", h=NH))

                def emit_p2(sb):
                    for part in range(4):
                        emit_p2_part(sb, part)

                def emit_normalize(j, drained):
                    # 1/rowsum on DVE, broadcast via a tiny f32r ones-
                    # matmul on the PE (GpSimd library-reload latency is
                    # ~8us, so it must stay off this path), then scale.
                    oaS_l, dn_l = drained
                    for pp in range(2):
                        rbp = psA.tile([128, 2 * SB], F32, name="rbp",
                                       tag="big")
                        for hl in range(2):
                            hg = 2 * pp + hl
                            rc = ep.tile([1, SB], F32, name="rc", tag="rc")
                            nc.vector.reciprocal_approx_fast(
                                rc[:], dn_l[hg][:])
                            rcb = ep.tile([1, SB], BF16, name="rcb", tag="rcb")
                            nc.vector.tensor_copy(rcb[:], rc[:])
                            nc.tensor.matmul(
                                rbp[0:HD, hl * SB:(hl + 1) * SB],
                                ones_bb[0:1, 0:HD],
                                rcb[:])
                        dstT = (outT01, outT23)[pp]
                        for hl in range(2):
                            hg = 2 * pp + hl
                            nc.vector.tensor_mul(
                                dstT[hl * HD:(hl + 1) * HD,
                                     j * SB:(j + 1) * SB],
                                oaS_l[hg][:],
                                rbp[0:HD, hl * SB:(hl + 1) * SB])

                emit_p2(0)
                drained = None
                p4q = []
                for j in range(NSB):
                    ktiles = list(range(4 * j + 4)) if causal else \
                        list(range(NST))
                    oacc = [oa.tile([VW, SB], F32, name=f"oa{j}_{h}",
                                    tag="oa") for h in range(NH)]
                    ets = {}
                    q0s = {}

                    def emit_av(i):
                        q0 = q0s[i]
                        et = ets.pop(i)
                        for hg in range(NH):
                            p, hl = hg // 2, hg % 2
                            nc.tensor.matmul(
                                oacc[hg][:, q0:SB],
                                v_sb[:, i * (NH * VW) + hg * VW:
                                     i * (NH * VW) + (hg + 1) * VW],
                                et[p][:, hl * SB + q0:hl * SB + SB],
                                start=(i == ktiles[0]), stop=(i == ktiles[-1]))

                    n_k = len(ktiles)
                    for idx, i in enumerate(ktiles):
                        q0 = 128 * (i - 4 * j) if (causal and i >= 4 * j) else 0
                        q0s[i] = q0
                        mt = None
                        if use_mask:
                            mt = mp.tile([128, SB], F32, name="mt", tag="mt")
                            nc.sync.dma_start(
                                mt[:], maskT[i * 128:(i + 1) * 128,
                                             j * SB:(j + 1) * SB])
                        pair_et = []
                        for p in range(2):
                            kTp = (kT01, kT23)[p]
                            qTp = (qT01, qT23)[p]
                            scp = psA.tile([128, 2 * SB], F32, name="scp",
                                           tag="big")
                            nc.tensor.matmul(
                                scp[:, q0:SB],
                                kTp[0:64, i * 128:(i + 1) * 128],
                                qTp[0:64, j * SB + q0:(j + 1) * SB])
                            nc.tensor.matmul(
                                scp[:, SB + q0:2 * SB],
                                kTp[64:128, i * 128:(i + 1) * 128],
                                qTp[64:128, j * SB + q0:(j + 1) * SB])
                            if use_mask:
                                nc.vector.tensor_add(scp[:, 0:SB],
                                                     scp[:, 0:SB], mt[:])
                                nc.vector.tensor_add(scp[:, SB:2 * SB],
                                                     scp[:, SB:2 * SB], mt[:])
                            et = etp.tile([128, 2 * SB], BF16, name="et",
                                          tag="et")
                            if q0:
                                src = scp.rearrange(
                                    "p (b c) -> p b c", b=2)[:, :, q0:]
                                dst = et.rearrange(
                                    "p (b c) -> p b c", b=2)[:, :, q0:]
                            else:
                                src, dst = scp[:], et[:]
                            nc.scalar.activation(dst, src, AF.Exp, scale=0.125)
                            if causal and i >= 4 * j:
                                # one DVE op gates both heads' diagonal
                                # strips (tri2 holds the gate twice)
                                et_r = et.rearrange(
                                    "p (b c) -> p b c", b=2)[:, :, q0:q0 + 128]
                                nc.vector.tensor_mul(
                                    et_r, et_r,
                                    tri2_sb.rearrange("p (b c) -> p b c", b=2))
                            pair_et.append(et)
                        ets[i] = pair_et
                        # staggered cross-phase injections: each lands well
                        # before its consumers so boundaries never stall
                        if idx == 1 and j > 0:
                            emit_normalize(j - 1, drained)
                            p4q = [(t4, dh) for t4 in range(4 * (j - 1),
                                                           4 * (j - 1) + 4)
                                   for dh in range(2)]
                        if idx >= 2 and p4q:
                            # one output-projection tile per k-tile keeps
                            # the PE continuously busy (HAM stays warm)
                            emit_p4_tile(*p4q.pop(0))
                        if j < NSB - 1 and n_k - 6 <= idx <= n_k - 3:
                            if n_k >= 8:
                                emit_p2_part(j + 1, idx - (n_k - 6))
                            elif idx == n_k - 3:
                                emit_p2(j + 1)
                        # software pipeline: attn@v for the previous k-tile
                        if idx > 0:
                            emit_av(ktiles[idx - 1])
                    while p4q:
                        emit_p4_tile(*p4q.pop(0))
                    emit_av(ktiles[-1])

                    # drain oacc to SBUF immediately (partition-aligned
                    # copies) so the next j's accumulators never wait
                    oaS_l, dn_l = [], []
                    for hg in range(NH):
                        oaS = ep.tile([HD, SB], F32, name="oaS", tag="oaS")
                        nc.vector.tensor_copy(oaS[:], oacc[hg][0:HD, :])
                        dn = ep.tile([1, SB], F32, name="dn", tag="dn")
                        nc.scalar.activation(dn[:], oacc[hg][HD:VW, :],
                                             AF.Copy)
                        oaS_l.append(oaS)
                        dn_l.append(dn)
                    drained = (oaS_l, dn_l)

                emit_normalize(NSB - 1, drained)
                for t4 in range(4 * (NSB - 1), 4 * NSB):
                    for dh in range(2):
                        emit_p4_tile(t4, dh)

    nc.finalize()
    return nc


_NC_CACHE = {}


def _get_nc(causal, use_mask):
    key = (causal, use_mask)
    if key not in _NC_CACHE:
        _NC_CACHE[key] = _build_nc(causal, use_mask)
    return _NC_CACHE[key]


def _prep_inputs(x, cos, sin, mask, w_kv_down, kv_norm_w, w_uk, w_ur, w_uv,
                 w_q_down, q_norm_w, w_uq, w_qr, w_o, use_mask):
    """Build the 8 per-core input maps (host-side shard + fold)."""
    import ml_dtypes as md
    f = np.float32
    x = np.asarray(x, f)
    cos = np.asarray(cos, f)
    sin = np.asarray(sin, f)
    w_kv_down = np.asarray(w_kv_down, f)
    w_q_down = np.asarray(w_q_down, f)
    kv_norm_w = np.asarray(kv_norm_w, f)
    q_norm_w = np.asarray(q_norm_w, f)
    w_uk_e = np.asarray(w_uk, f) * kv_norm_w[:, None]
    w_ur_e = np.asarray(w_ur, f) * kv_norm_w[:, None]
    w_uv_e = np.asarray(w_uv, f) * kv_norm_w[:, None]
    w_uq_e = np.asarray(w_uq, f) * q_norm_w[:, None]
    w_qr_e = np.asarray(w_qr, f) * q_norm_w[:, None]
    w_o = np.asarray(w_o, f)

    # shared rearrangements
    wkv = np.ascontiguousarray(
        w_kv_down.reshape(NKC, KC, R).transpose(1, 0, 2).reshape(KC, D))
    wq = np.ascontiguousarray(
        w_q_down.reshape(NKC, KC, R).transpose(1, 0, 2).reshape(KC, D))
    cosT = np.ascontiguousarray(cos.T)                 # [32, S]
    sinT = np.ascontiguousarray(sin.T)
    sinSg = np.concatenate([-sinT[:DR // 2], sinT[DR // 2:]], axis=0)
    one32 = np.ones((DR, S), np.float32)
    zero32 = np.zeros((DR, S), np.float32)
    # pair-tensor rope tables: nope rows pass through (cos=1, sin=0)
    cosPt = np.ascontiguousarray(
        np.concatenate([one32, cosT, one32, cosT], axis=0)).astype(md.bfloat16)
    sinPt = np.ascontiguousarray(
        np.concatenate([zero32, sinSg, zero32, sinSg], axis=0)).astype(md.bfloat16)
    # rope shift permutation within each head's 32 cols
    perm = np.concatenate([np.arange(16, 32), np.arange(0, 16)])

    xTb = [np.ascontiguousarray(x[b].T).astype(md.bfloat16) for b in range(B)]
    maskT8 = None
    if use_mask:
        m = np.asarray(mask, f).reshape(S, S)
        maskT8 = np.ascontiguousarray(m.T) * 8.0

    in_maps = []
    z32 = np.zeros((R, DN), np.float32)
    for core in range(NCORES):
        b, g = core // 4, core % 4
        cs = slice(g * NH * DN, (g + 1) * NH * DN)      # 128-wide col slice
        vs = slice(g * NH * HD, (g + 1) * NH * HD)      # 256-wide
        uk_l = w_uk_e[:, cs].reshape(R, NH, DN)
        ur_l = w_ur_e[:, cs].reshape(R, NH, DR)
        urs_l = ur_l[:, :, perm]
        uq_l = w_uq_e[:, cs].reshape(R, NH, DN)
        qr_l = w_qr_e[:, cs].reshape(R, NH, DR)
        qrs_l = qr_l[:, :, perm]
        # pair layout: [he_nope | he_rope | ho_nope | ho_rope] per 128 cols
        def pair(nope, rope):
            cols = []
            for h in range(NH):
                cols += [nope[:, h], rope[:, h]]
            return np.ascontiguousarray(np.concatenate(cols, axis=1))
        def pair_sh(sh):
            cols = []
            for h in range(NH):
                cols += [z32, sh[:, h]]
            return np.ascontiguousarray(np.concatenate(cols, axis=1))
        wo_loc = w_o[g * NH * HD:(g + 1) * NH * HD]     # [256, D]
        wo_r = np.ascontiguousarray(
            wo_loc.reshape(2, KC, D).transpose(1, 0, 2).reshape(KC, 2 * D)
        ).astype(md.bfloat16)
        m_ = {
            "xT": xTb[b],
            "wkv": wkv.astype(md.bfloat16), "wq": wq.astype(md.bfloat16),
            "kb": pair(uk_l, ur_l).astype(md.bfloat16),
            "ksh": pair_sh(urs_l).astype(md.bfloat16),
            "qb": pair(uq_l, qr_l).astype(md.bfloat16),
            "qsh": pair_sh(qrs_l).astype(md.bfloat16),
            "uv": np.ascontiguousarray(w_uv_e[:, vs]).astype(md.bfloat16),
            "wo": wo_r,
            "cosP": cosPt, "sinP": sinPt,
        }
        if use_mask:
            m_["maskT"] = maskT8
        in_maps.append(m_)
    return in_maps


def _classify_mask(mask):
    m = np.asarray(mask, np.float32).reshape(S, S)
    if not np.any(m):
        return False, False          # dense, no mask
    causal_ref = np.where(
        np.tril(np.ones((S, S), dtype=bool)), np.float32(0.0),
        np.float32(-1e9))
    if np.array_equal(m, causal_ref):
        return True, False           # structural causal
    return False, True               # generic additive mask


LAST_RESULTS = None


def kernel(**inputs):
    global LAST_RESULTS
    from concourse.bass_utils import run_bass_kernel_spmd
    causal, use_mask = _classify_mask(inputs["mask"])
    nc = _get_nc(causal, use_mask)
    in_maps = _prep_inputs(
        inputs["x"], inputs["cos"], inputs["sin"], inputs["mask"],
        inputs["w_kv_down"], inputs["kv_norm_w"], inputs["w_uk"],
        inputs["w_ur"], inputs["w_uv"], inputs["w_q_down"],
        inputs["q_norm_w"], inputs["w_uq"], inputs["w_qr"], inputs["w_o"],
        use_mask)
    res = run_bass_kernel_spmd(nc, in_maps, list(range(NCORES)))
    LAST_RESULTS = res
    out = np.empty((B, S, D), np.float32)
    for b in range(B):
        acc = np.zeros((S, D), np.float32)
        for g in range(4):
            acc += np.asarray(res.results[4 * b + g]["y"]).astype(np.float32)
        out[b] = acc
    return out


# revision 29
# speedup vs baseline: 1.0500x; 1.0097x over previous
"""DeepSeek-MLA forward kernel for 8 Trainium2 NeuronCores (Bass/Tile).

Sharding: core c -> batch b = c // 4, head-group g = c % 4 (4 of 16 heads).
Each core computes its batch's down-projections (replicated x4 within the
batch group), its 4 heads' attention, and a partial output projection
y_part = out_heads_local @ w_o_local (stored bf16).  The host sums the 4
partials per batch (fp32) and stacks the 2 batches.

v2 design notes (vs the 344us baseline):
- P1 is k-chunk-outer: xT is loaded once as 16 [128, S] chunks (4KB DMA
  rows), accumulating all 4 s-blocks x {kv,q} in 8 PSUM banks, so the PE
  streams at DMA arrival rate with no re-loads.
- rmsnorm sum-of-squares via a ones[128x128] matmul (output broadcast to
  all partitions), reciprocal on DVE (reciprocal_approx_fast), sqrt on ACT.
  Scalar engine table loads: Sqrt once, then Exp once - no thrashing.
- P3 scores are computed per 2-head pair into [128, 2*SB] PSUM tiles so
  exp runs as one ACT op per pair (amortizes the ~200-cycle ACT overhead).
  The attn@v matmuls for k-tile i are emitted after the scores of k-tile
  i+1 (software pipelining) so the PE never waits on exp.
- Softmax epilogue: DVE reciprocal + gpsimd partition_broadcast + DVE mul
  (no Ln/Exp activation-table swaps).
- P4 (output projection) is interleaved into the attention j-loop so its
  matmuls fill PE slack while ACT catches up; partials stored as bf16.
"""

import os
import sys

import numpy as np

for _p in ("/opt/trn_rl_repo", "/root/.axon_site/_ro/trn_rl_repo"):
    if os.path.isdir(_p) and _p not in sys.path:
        sys.path.insert(0, _p)

import concourse.bass as bass
import concourse.mybir as mybir
import concourse.tile as tile
from concourse import bacc

B, S, D, H, DN, DR, R = 2, 2048, 2048, 16, 32, 32, 128
HD = DN + DR  # 64
NCORES = 8
NH = 4          # heads per core
SB = 512        # s-block (psum bank width in f32)
NSB = S // SB   # 4
ST = 128        # s-tile
NST = S // ST   # 16
KC = 128        # contraction chunk
NKC = D // KC   # 16
VW = HD + 1     # v columns incl. ones column (65)
F32 = mybir.dt.float32
BF16 = mybir.dt.bfloat16


def _build_nc(causal: bool, use_mask: bool):
    nc = bacc.Bacc("TRN2", target_bir_lowering=False, debug=False,
                   num_devices=NCORES)

    xT = nc.dram_tensor("xT", [D, S], BF16, kind="ExternalInput").ap()
    wkv = nc.dram_tensor("wkv", [KC, D], BF16, kind="ExternalInput").ap()
    wq = nc.dram_tensor("wq", [KC, D], BF16, kind="ExternalInput").ap()
    kb = nc.dram_tensor("kb", [R, 2 * KC], BF16, kind="ExternalInput").ap()
    ksh = nc.dram_tensor("ksh", [R, 2 * KC], BF16, kind="ExternalInput").ap()
    qb = nc.dram_tensor("qb", [R, 2 * KC], BF16, kind="ExternalInput").ap()
    qsh = nc.dram_tensor("qsh", [R, 2 * KC], BF16, kind="ExternalInput").ap()
    uv = nc.dram_tensor("uv", [R, NH * HD], BF16, kind="ExternalInput").ap()
    wo = nc.dram_tensor("wo", [KC, 2 * D], BF16, kind="ExternalInput").ap()
    cosP = nc.dram_tensor("cosP", [128, S], BF16, kind="ExternalInput").ap()
    sinP = nc.dram_tensor("sinP", [128, S], BF16, kind="ExternalInput").ap()
    maskT = None
    if use_mask:
        maskT = nc.dram_tensor("maskT", [S, S], F32, kind="ExternalInput").ap()
    y = nc.dram_tensor("y", [S, D], BF16, kind="ExternalOutput").ap()

    AF = mybir.ActivationFunctionType
    ALU = mybir.AluOpType

    with tile.TileContext(nc) as tc:
        from contextlib import ExitStack
        with ExitStack() as ctx:
            stat = ctx.enter_context(tc.tile_pool(name="static", bufs=1))
            # persistent SBUF tensors
            ckvT = stat.tile([R, S], BF16, name="ckvT")
            cqT = stat.tile([R, S], BF16, name="cqT")
            kT01 = stat.tile([128, S], BF16, name="kT01")
            kT23 = stat.tile([128, S], BF16, name="kT23")
            qT01 = stat.tile([128, S], BF16, name="qT01")
            qT23 = stat.tile([128, S], BF16, name="qT23")
            v_sb = stat.tile([128, NST * NH * VW], BF16, name="v_sb")
            outT01 = stat.tile([128, S], BF16, name="outT01")
            outT23 = stat.tile([128, S], BF16, name="outT23")
            wkv_sb = stat.tile([KC, D], BF16, name="wkv_sb")
            wq_sb = stat.tile([KC, D], BF16, name="wq_sb")
            kb_sb = stat.tile([R, 2 * KC], BF16, name="kb_sb")
            ksh_sb = stat.tile([R, 2 * KC], BF16, name="ksh_sb")
            qb_sb = stat.tile([R, 2 * KC], BF16, name="qb_sb")
            qsh_sb = stat.tile([R, 2 * KC], BF16, name="qsh_sb")
            uv_sb = stat.tile([R, NH * HD], BF16, name="uv_sb")
            wo_sb = stat.tile([KC, 2 * D], BF16, name="wo_sb")
            cosP_sb = stat.tile([128, S], BF16, name="cosP_sb")
            sinP_sb = stat.tile([128, S], BF16, name="sinP_sb")
            ones_bb = stat.tile([128, 128], BF16, name="ones_bb")
            tri2_sb = stat.tile([128, 256], BF16, name="tri2_sb")
            onesf_sb = stat.tile([128, 64], F32, name="onesf_sb")

            # P1-critical loads first so the PE can start ASAP (weight
            # pieces interleaved with the first x chunks).

            # ---------------- Phase 1: c_kv^T, c_q^T + RMS norm ----------
            with tc.tile_pool(name="p1x", bufs=1) as p1x, \
                 tc.tile_pool(name="p1ps", bufs=8, space="PSUM") as p1ps, \
                 tc.tile_pool(name="p1t", bufs=3) as p1t:
                xch = [p1x.tile([128, S], BF16, name=f"xch{k}", tag=f"x{k}")
                       for k in range(NKC)]
                for k in range(NKC):
                    if k < 4:
                        pc = slice(k * SB, (k + 1) * SB)
                        nc.sync.dma_start(wkv_sb[:, pc], wkv[:, pc])
                        nc.sync.dma_start(wq_sb[:, pc], wq[:, pc])
                    nc.sync.dma_start(xch[k][:], xT[k * KC:(k + 1) * KC, :])
                # remaining static loads (after the P1-critical stream)
                nc.sync.dma_start(kb_sb[:], kb)
                nc.sync.dma_start(ksh_sb[:], ksh)
                nc.sync.dma_start(qb_sb[:], qb)
                nc.sync.dma_start(qsh_sb[:], qsh)
                nc.sync.dma_start(uv_sb[:], uv)
                nc.sync.dma_start(cosP_sb[:], cosP)
                nc.sync.dma_start(sinP_sb[:], sinP)
                nc.sync.dma_start(wo_sb[:], wo)
                nc.gpsimd.memset(ones_bb[:], 1.0)
                nc.gpsimd.memset(onesf_sb[:], 1.0)
                # tri[p, f] = 1.0 if p <= f else 0.0 (keep-lower-triangle
                # gate for diagonal score strips in k-major layout),
                # stored twice side by side so one DVE op covers a pair
                nc.gpsimd.memset(tri2_sb[:], 1.0)
                for _h in range(2):
                    nc.gpsimd.affine_select(
                        out=tri2_sb[:, _h * 128:(_h + 1) * 128],
                        in_=tri2_sb[:, _h * 128:(_h + 1) * 128],
                        compare_op=ALU.is_ge, fill=0.0, base=0,
                        channel_multiplier=-1, pattern=[[1, 128]])
                # ones column of v (col 64 of each 65-wide block)
                v_blocks = v_sb.rearrange("p (t h w) -> p t h w", t=NST, h=NH)
                nc.vector.tensor_copy(
                    v_blocks[:, :, :, HD:VW],
                    onesf_sb.rearrange("p (t h w) -> p t h w", t=NST, h=NH))

                cps = {}
                for sb in range(NSB):
                    for t, _ in ((0, None), (1, None)):
                        cps[(sb, t)] = p1ps.tile(
                            [128, SB], F32, name=f"cps{sb}_{t}", tag="cps")
                for k in range(NKC - 1):
                    for t, wsb in ((0, wkv_sb), (1, wq_sb)):
                        for sb in range(NSB):
                            nc.tensor.matmul(
                                cps[(sb, t)][:],
                                wsb[:, k * KC:(k + 1) * KC],
                                xch[k][:, sb * SB:(sb + 1) * SB],
                                start=(k == 0), stop=False)
                # last k-chunk + rmsnorm drain, phase-ordered so the
                # in-order PE never head-of-line blocks on a later
                # block's ones-matmul: block 0's full chain first (P2(0)
                # depends on it), then all copies/squares, then the
                # ones-matmuls, then the reciprocal/sqrt/scale tails.
                k = NKC - 1

                def k15(sb):
                    for t, wsb in ((0, wkv_sb), (1, wq_sb)):
                        nc.tensor.matmul(
                            cps[(sb, t)][:],
                            wsb[:, k * KC:(k + 1) * KC],
                            xch[k][:, sb * SB:(sb + 1) * SB],
                            start=False, stop=True)

                sqts, mss = {}, {}

                def drain_head(sb):
                    sl = slice(sb * SB, (sb + 1) * SB)
                    for t, cT in ((0, ckvT), (1, cqT)):
                        nc.scalar.activation(cT[:, sl], cps[(sb, t)][:],
                                             AF.Copy)
                        sqt = p1t.tile([128, SB], BF16, name="sqt",
                                       tag="sqt", bufs=8)
                        nc.vector.tensor_mul(sqt[:], cT[:, sl], cT[:, sl])
                        sqts[(sb, t)] = sqt

                def drain_ms(sb):
                    for t in range(2):
                        ms = p1ps.tile([128, SB], F32, name="ms", tag="cps")
                        nc.tensor.matmul(ms[:], ones_bb[:], sqts[(sb, t)][:])
                        mss[(sb, t)] = ms

                def drain_tail(sb):
                    sl = slice(sb * SB, (sb + 1) * SB)
                    for t, cT in ((0, ckvT), (1, cqT)):
                        u = p1t.tile([128, SB], F32, name="u", tag="u")
                        nc.vector.reciprocal_approx_fast(u[:], mss[(sb, t)][:])
                        rstd = p1t.tile([128, SB], F32, name="rstd",
                                        tag="rstd")
                        nc.scalar.activation(rstd[:], u[:], AF.Sqrt,
                                             scale=float(R))
                        nc.vector.tensor_mul(cT[:, sl], cT[:, sl], rstd[:])

                k15(0)
                drain_head(0)
                drain_ms(0)
                drain_tail(0)
                for sb in range(1, NSB):
                    k15(sb)
                    drain_head(sb)
                for sb in range(1, NSB):
                    drain_ms(sb)
                for sb in range(1, NSB):
                    drain_tail(sb)

            # ---------------- Phases 2+3+4 fused over s-blocks -----------
            with tc.tile_pool(name="psA", bufs=2, space="PSUM") as psA, \
                 tc.tile_pool(name="oa", bufs=4, space="PSUM") as oa, \
                 tc.tile_pool(name="etp", bufs=6) as etp, \
                 tc.tile_pool(name="vt", bufs=6) as vt, \
                 tc.tile_pool(name="yb", bufs=4) as yb, \
                 tc.tile_pool(name="ep", bufs=4) as ep, \
                 tc.tile_pool(name="mp", bufs=3) as mp:

                F32R = mybir.dt.float32r

                def emit_p4_tile(t4, dh):
                    yp = psA.tile([128, 2 * SB], F32, name="yp", tag="big")
                    for c, oT in ((0, outT01), (1, outT23)):
                        for half in range(2):
                            dlo = dh * 1024 + half * SB
                            nc.tensor.matmul(
                                yp[:, half * SB:(half + 1) * SB],
                                oT[:, t4 * ST:(t4 + 1) * ST],
                                wo_sb[:, c * D + dlo:c * D + dlo + SB],
                                start=(c == 0), stop=(c == 1))
                    ysb = yb.tile([128, 2 * SB], BF16, name="ysb", tag="y")
                    # full drain on DVE: ACT must stay exp-only in steady
                    # state (a scalar-side copy here saturates ACT and
                    # stalls the score ring ~730ns every P4-carrying tile)
                    nc.vector.tensor_copy(ysb[:], yp[:])
                    nc.sync.dma_start(
                        y[t4 * ST:(t4 + 1) * ST,
                          dh * 1024:(dh + 1) * 1024], ysb[:])

                def emit_p2_part(sb, part):
                    # one quarter of the up-projection + rope work for
                    # s-block sb (q pairs first, then kv), plus one v tile.
                    # Pair tensors: rows [he_nope|he_rope|ho_nope|ho_rope];
                    # cosP rows are 1.0 (sinP rows 0.0) on nope rows so one
                    # fused 3-op rope pass covers nope+rope together.
                    sl = slice(sb * SB, (sb + 1) * SB)
                    cT, wb, wsh, dsts = (
                        (cqT, qb_sb, qsh_sb, (qT01, qT23)),
                        (ckvT, kb_sb, ksh_sb, (kT01, kT23)))[part // 2]
                    p = part % 2
                    quad = psA.tile([128, 2 * SB], F32, name="p2q", tag="big")
                    nc.tensor.matmul(quad[:, 0:SB],
                                     wb[:, p * KC:(p + 1) * KC], cT[:, sl])
                    nc.tensor.matmul(quad[:, SB:2 * SB],
                                     wsh[:, p * KC:(p + 1) * KC], cT[:, sl])
                    t1 = vt.tile([128, SB], BF16, name="t1", tag="t")
                    t2 = vt.tile([128, SB], BF16, name="t2", tag="t")
                    nc.vector.tensor_mul(t1[:], quad[:, 0:SB], cosP_sb[:, sl])
                    nc.vector.tensor_mul(t2[:], quad[:, SB:2 * SB],
                                         sinP_sb[:, sl])
                    nc.vector.tensor_add(dsts[p][:, sl], t1[:], t2[:])
                    t4 = 4 * sb + part
                    vq = psA.tile([128, 2 * SB], F32, name="vq", tag="big")
                    nc.tensor.matmul(vq[:, 0:NH * HD],
                                     ckvT[:, t4 * ST:(t4 + 1) * ST], uv_sb[:])
                    nc.vector.tensor_copy(
                        v_blocks[:, t4, :, 0:HD],
                        vq[:, 0:NH * HD].rearrange("p (h d) -> p h d", h=NH))

                def emit_p2(sb):
                    for part in range(4):
                        emit_p2_part(sb, part)

                def emit_normalize(j, drained):
                    # 1/rowsum on DVE, broadcast via a tiny f32r ones-
                    # matmul on the PE (GpSimd library-reload latency is
                    # ~8us, so it must stay off this path), then scale.
                    oaS_l, dn_l = drained
                    for pp in range(2):
                        rbp = psA.tile([128, 2 * SB], F32, name="rbp",
                                       tag="big")
                        for hl in range(2):
                            hg = 2 * pp + hl
                            rc = ep.tile([1, SB], F32, name="rc", tag="rc")
                            nc.vector.reciprocal_approx_fast(
                                rc[:], dn_l[hg][:])
                            rcb = ep.tile([1, SB], BF16, name="rcb", tag="rcb")
                            nc.vector.tensor_copy(rcb[:], rc[:])
                            nc.tensor.matmul(
                                rbp[0:HD, hl * SB:(hl + 1) * SB],
                                ones_bb[0:1, 0:HD],
                                rcb[:])
                        dstT = (outT01, outT23)[pp]
                        for hl in range(2):
                            hg = 2 * pp + hl
                            nc.vector.tensor_mul(
                                dstT[hl * HD:(hl + 1) * HD,
                                     j * SB:(j + 1) * SB],
                                oaS_l[hg][:],
                                rbp[0:HD, hl * SB:(hl + 1) * SB])

                emit_p2(0)
                drained = None
                p4q = []
                for j in range(NSB):
                    ktiles = list(range(4 * j + 4)) if causal else \
                        list(range(NST))
                    oacc = [oa.tile([VW, SB], F32, name=f"oa{j}_{h}",
                                    tag="oa") for h in range(NH)]
                    ets = {}
                    q0s = {}

                    def emit_av(i):
                        q0 = q0s[i]
                        et = ets.pop(i)
                        for hg in range(NH):
                            p, hl = hg // 2, hg % 2
                            nc.tensor.matmul(
                                oacc[hg][:, q0:SB],
                                v_sb[:, i * (NH * VW) + hg * VW:
                                     i * (NH * VW) + (hg + 1) * VW],
                                et[p][:, hl * SB + q0:hl * SB + SB],
                                start=(i == ktiles[0]), stop=(i == ktiles[-1]))

                    n_k = len(ktiles)
                    for idx, i in enumerate(ktiles):
                        q0 = 128 * (i - 4 * j) if (causal and i >= 4 * j) else 0
                        q0s[i] = q0
                        mt = None
                        if use_mask:
                            mt = mp.tile([128, SB], F32, name="mt", tag="mt")
                            nc.sync.dma_start(
                                mt[:], maskT[i * 128:(i + 1) * 128,
                                             j * SB:(j + 1) * SB])
                        pair_et = []
                        for p in range(2):
                            kTp = (kT01, kT23)[p]
                            qTp = (qT01, qT23)[p]
                            scp = psA.tile([128, 2 * SB], F32, name="scp",
                                           tag="big")
                            nc.tensor.matmul(
                                scp[:, q0:SB],
                                kTp[0:64, i * 128:(i + 1) * 128],
                                qTp[0:64, j * SB + q0:(j + 1) * SB])
                            nc.tensor.matmul(
                                scp[:, SB + q0:2 * SB],
                                kTp[64:128, i * 128:(i + 1) * 128],
                                qTp[64:128, j * SB + q0:(j + 1) * SB])
                            if use_mask:
                                nc.vector.tensor_add(scp[:, 0:SB],
                                                     scp[:, 0:SB], mt[:])
                                nc.vector.tensor_add(scp[:, SB:2 * SB],
                                                     scp[:, SB:2 * SB], mt[:])
                            et = etp.tile([128, 2 * SB], BF16, name="et",
                                          tag="et")
                            if q0:
                                src = scp.rearrange(
                                    "p (b c) -> p b c", b=2)[:, :, q0:]
                                dst = et.rearrange(
                                    "p (b c) -> p b c", b=2)[:, :, q0:]
                            else:
                                src, dst = scp[:], et[:]
                            nc.scalar.activation(dst, src, AF.Exp, scale=0.125)
                            if causal and i >= 4 * j:
                                # one DVE op gates both heads' diagonal
                                # strips (tri2 holds the gate twice)
                                et_r = et.rearrange(
                                    "p (b c) -> p b c", b=2)[:, :, q0:q0 + 128]
                                nc.vector.tensor_mul(
                                    et_r, et_r,
                                    tri2_sb.rearrange("p (b c) -> p b c", b=2))
                            pair_et.append(et)
                        ets[i] = pair_et
                        # staggered cross-phase injections: each lands well
                        # before its consumers so boundaries never stall
                        if idx == 1 and j > 0:
                            emit_normalize(j - 1, drained)
                            p4q = [(t4, dh) for t4 in range(4 * (j - 1),
                                                           4 * (j - 1) + 4)
                                   for dh in range(2)]
                        if idx >= 2 and p4q:
                            # one output-projection tile per k-tile keeps
                            # the PE continuously busy (HAM stays warm)
                            emit_p4_tile(*p4q.pop(0))
                        if j < NSB - 1 and n_k - 6 <= idx <= n_k - 3:
                            if n_k >= 8:
                                emit_p2_part(j + 1, idx - (n_k - 6))
                            elif idx == n_k - 3:
                                emit_p2(j + 1)
                        # software pipeline: attn@v for the previous k-tile
                        if idx > 0:
                            emit_av(ktiles[idx - 1])
                    while p4q:
                        emit_p4_tile(*p4q.pop(0))
                    emit_av(ktiles[-1])

                    # drain oacc to SBUF immediately (partition-aligned
                    # copies) so the next j's accumulators never wait
                    oaS_l, dn_l = [], []
                    for hg in range(NH):
                        oaS = ep.tile([HD, SB], F32, name="oaS", tag="oaS")
                        nc.vector.tensor_copy(oaS[:], oacc[hg][0:HD, :])
                        dn = ep.tile([1, SB], F32, name="dn", tag="dn")
                        nc.scalar.activation(dn[:], oacc[hg][HD:VW, :],
                                             AF.Copy)
                        oaS_l.append(oaS)
                        dn_l.append(dn)
                    drained = (oaS_l, dn_l)

                emit_normalize(NSB - 1, drained)
                for t4 in range(4 * (NSB - 1), 4 * NSB):
                    for dh in range(2):
                        emit_p4_tile(t4, dh)

    nc.finalize()
    return nc


_NC_CACHE = {}


def _get_nc(causal, use_mask):
    key = (causal, use_mask)
    if key not in _NC_CACHE:
        _NC_CACHE[key] = _build_nc(causal, use_mask)
    return _NC_CACHE[key]


def _prep_inputs(x, cos, sin, mask, w_kv_down, kv_norm_w, w_uk, w_ur, w_uv,
                 w_q_down, q_norm_w, w_uq, w_qr, w_o, use_mask):
    """Build the 8 per-core input maps (host-side shard + fold)."""
    import ml_dtypes as md
    f = np.float32
    x = np.asarray(x, f)
    cos = np.asarray(cos, f)
    sin = np.asarray(sin, f)
    w_kv_down = np.asarray(w_kv_down, f)
    w_q_down = np.asarray(w_q_down, f)
    kv_norm_w = np.asarray(kv_norm_w, f)
    q_norm_w = np.asarray(q_norm_w, f)
    w_uk_e = np.asarray(w_uk, f) * kv_norm_w[:, None]
    w_ur_e = np.asarray(w_ur, f) * kv_norm_w[:, None]
    w_uv_e = np.asarray(w_uv, f) * kv_norm_w[:, None]
    w_uq_e = np.asarray(w_uq, f) * q_norm_w[:, None]
    w_qr_e = np.asarray(w_qr, f) * q_norm_w[:, None]
    w_o = np.asarray(w_o, f)

    # shared rearrangements
    wkv = np.ascontiguousarray(
        w_kv_down.reshape(NKC, KC, R).transpose(1, 0, 2).reshape(KC, D))
    wq = np.ascontiguousarray(
        w_q_down.reshape(NKC, KC, R).transpose(1, 0, 2).reshape(KC, D))
    cosT = np.ascontiguousarray(cos.T)                 # [32, S]
    sinT = np.ascontiguousarray(sin.T)
    sinSg = np.concatenate([-sinT[:DR // 2], sinT[DR // 2:]], axis=0)
    one32 = np.ones((DR, S), np.float32)
    zero32 = np.zeros((DR, S), np.float32)
    # pair-tensor rope tables: nope rows pass through (cos=1, sin=0)
    cosPt = np.ascontiguousarray(
        np.concatenate([one32, cosT, one32, cosT], axis=0)).astype(md.bfloat16)
    sinPt = np.ascontiguousarray(
        np.concatenate([zero32, sinSg, zero32, sinSg], axis=0)).astype(md.bfloat16)
    # rope shift permutation within each head's 32 cols
    perm = np.concatenate([np.arange(16, 32), np.arange(0, 16)])

    xTb = [np.ascontiguousarray(x[b].T).astype(md.bfloat16) for b in range(B)]
    maskT8 = None
    if use_mask:
        m = np.asarray(mask, f).reshape(S, S)
        maskT8 = np.ascontiguousarray(m.T) * 8.0

    in_maps = []
    z32 = np.zeros((R, DN), np.float32)
    for core in range(NCORES):
        b, g = core // 4, core % 4
        cs = slice(g * NH * DN, (g + 1) * NH * DN)      # 128-wide col slice
        vs = slice(g * NH * HD, (g + 1) * NH * HD)      # 256-wide
        uk_l = w_uk_e[:, cs].reshape(R, NH, DN)
        ur_l = w_ur_e[:, cs].reshape(R, NH, DR)
        urs_l = ur_l[:, :, perm]
        uq_l = w_uq_e[:, cs].reshape(R, NH, DN)
        qr_l = w_qr_e[:, cs].reshape(R, NH, DR)
        qrs_l = qr_l[:, :, perm]
        # pair layout: [he_nope | he_rope | ho_nope | ho_rope] per 128 cols
        def pair(nope, rope):
            cols = []
            for h in range(NH):
                cols += [nope[:, h], rope[:, h]]
            return np.ascontiguousarray(np.concatenate(cols, axis=1))
        def pair_sh(sh):
            cols = []
            for h in range(NH):
                cols += [z32, sh[:, h]]
            return np.ascontiguousarray(np.concatenate(cols, axis=1))
        wo_loc = w_o[g * NH * HD:(g + 1) * NH * HD]     # [256, D]
        wo_r = np.ascontiguousarray(
            wo_loc.reshape(2, KC, D).transpose(1, 0, 2).reshape(KC, 2 * D)
        ).astype(md.bfloat16)
        m_ = {
            "xT": xTb[b],
            "wkv": wkv.astype(md.bfloat16), "wq": wq.astype(md.bfloat16),
            "kb": pair(uk_l, ur_l).astype(md.bfloat16),
            "ksh": pair_sh(urs_l).astype(md.bfloat16),
            "qb": pair(uq_l, qr_l).astype(md.bfloat16),
            "qsh": pair_sh(qrs_l).astype(md.bfloat16),
            "uv": np.ascontiguousarray(w_uv_e[:, vs]).astype(md.bfloat16),
            "wo": wo_r,
            "cosP": cosPt, "sinP": sinPt,
        }
        if use_mask:
            m_["maskT"] = maskT8
        in_maps.append(m_)
    return in_maps


def _classify_mask(mask):
    m = np.asarray(mask, np.float32).reshape(S, S)
    if not np.any(m):
        return False, False          # dense, no mask
    causal_ref = np.where(
        np.tril(np.ones((S, S), dtype=bool)), np.float32(0.0),
        np.float32(-1e9))
    if np.array_equal(m, causal_ref):
        return True, False           # structural causal
    return False, True               # generic additive mask


LAST_RESULTS = None


def kernel(**inputs):
    global LAST_RESULTS
    from concourse.bass_utils import run_bass_kernel_spmd
    causal, use_mask = _classify_mask(inputs["mask"])
    nc = _get_nc(causal, use_mask)
    in_maps = _prep_inputs(
        inputs["x"], inputs["cos"], inputs["sin"], inputs["mask"],
        inputs["w_kv_down"], inputs["kv_norm_w"], inputs["w_uk"],
        inputs["w_ur"], inputs["w_uv"], inputs["w_q_down"],
        inputs["q_norm_w"], inputs["w_uq"], inputs["w_qr"], inputs["w_o"],
        use_mask)
    res = run_bass_kernel_spmd(nc, in_maps, list(range(NCORES)))
    LAST_RESULTS = res
    out = np.empty((B, S, D), np.float32)
    for b in range(B):
        acc = np.zeros((S, D), np.float32)
        for g in range(4):
            acc += np.asarray(res.results[4 * b + g]["y"]).astype(np.float32)
        out[b] = acc
    return out
